# revision 1
# baseline (speedup 1.0000x reference)
"""GAT (2-layer) on 8 NeuronCores — Bass/Tile kernel.

Strategy (dst-sharded graph parallel):
  - Each core owns 12500 destination nodes, split into 6 sub-shards
    (round-robin over the degree-sorted order) so each sub-shard's quad
    table stays within dma_gather's int16 index range.
  - Slot layout: degree-sorted 128-dst tiles, per-tile slot capacity D from
    a multiple-of-4 grid. Slots are grouped 4-at-a-time into "quads"; the
    halo table holds one 512B row per distinct quad (4 x 64 fp16 features),
    so each gather descriptor moves 512B (no sub-512B DMA penalty).
  - Launch A: per-core Wh1^T = (x W1 + b)^T + attention scalars s_i/s_j.
  - Host between launches: packs quad tables from device-computed Wh
    (fp16), expands s_j per slot (f32, with -60 at pad slots, which
    doubles as the softmax pad mask), folds bA and a global logit shift
    into s_i (softmax is shift-invariant; keeps exp inside fp16 range
    without a per-tile max pass). Index-only work
    plus value repacking; all model FLOPs run on device.
  - Launch B (x2, one per GAT layer): wide quad dma_gathers, segment
    softmax over the slot axis (leaky-relu on DVE, exp broadcast-expanded
    to fp16 on Act), 2x-rate fp16 multiply + in-place halving-tree slot
    sum, alpha-normalize + leaky, per-tile PE transpose into shared PSUM
    chunks, epilogue matmul with the next layer's weights.
"""

import bisect
import dataclasses
import hashlib
import numpy as np

import concourse.bacc as bacc
import concourse.tile as tile
from concourse import bass, mybir, bass_utils
from concourse.masks import make_identity

F32 = mybir.dt.float32
F16 = mybir.dt.float16
I16 = mybir.dt.int16

N_NODES = 100000
N_CORES = 8
DPC = N_NODES // N_CORES
F = 64
IN_C = 128
NSUB = 6
WQMAX = 32  # quad columns per gather call (128 slots)
SMAX = 4 * WQMAX  # slot columns per gather call
GRID = [4, 8, 12, 16, 20, 24, 28, 32, 36, 40, 44, 48, 56, 64, 80, 96, 128]
CHT = 4  # tiles per epilogue chunk (512 dsts; fp32 matmul N<=512)
PAD_SJ = -60.0
ALPHA = 0.2


@dataclasses.dataclass
class Schedule:
    n_tiles: int
    w_total: int  # slot columns
    wq_total: int  # quad columns
    rsubq: int  # quad-table rows per sub-shard
    tiles: list  # per global tile: (sub, D)
    calls: list  # (sub, qcol0, Wq, parts) ; parts: [(tile0, D, ntc, lq)]
    perms: list  # per core: int64 [n_tiles*128], local dst or -1
    idx16: list  # per core: int16 [128, 8*wq_total]
    qrows: list  # per core: (row_ids, nodes[R,4] int32 with -1 pads)
    edges: list  # per core: (e_p, e_col, e_src) for sj_slot expansion


def _grid_up(x):
    return GRID[bisect.bisect_left(GRID, max(1, int(x)))]


def build_schedule(edge_index: np.ndarray) -> Schedule:
    src = np.asarray(edge_index[0], dtype=np.int64)
    dst = np.asarray(edge_index[1], dtype=np.int64)
    E = src.shape[0]
    order = np.argsort(dst, kind="stable")
    src_s = src[order]
    dst_s = dst[order]
    deg_all = np.bincount(dst, minlength=N_NODES)
    starts_all = np.concatenate([[0], np.cumsum(deg_all)])
    k_s = np.arange(E) - starts_all[dst_s]

    # ascending-degree round robin; the LAST sub-shard is reversed
    # (descending) so the program's final tiles are small and the epilogue
    # tail is short
    core_subs = []
    for c in range(N_CORES):
        deg = deg_all[c * DPC : (c + 1) * DPC]
        rank = np.argsort(deg, kind="stable")
        subs = [rank[s::NSUB] for s in range(NSUB)]
        subs[NSUB - 1] = subs[NSUB - 1][::-1]
        core_subs.append(subs)

    def _blockify_s(s, lst, nt):
        # partial tile holds the low-degree end: front-pad ascending subs,
        # back-pad the reversed (descending) last sub
        block = np.full(nt * 128, -1, np.int64)
        if s == NSUB - 1:
            block[: len(lst)] = lst
        else:
            block[nt * 128 - len(lst) :] = lst
        return block

    tiles = []
    sub_nt = []
    for s in range(NSUB):
        nt = max(-(-len(core_subs[c][s]) // 128) for c in range(N_CORES))
        sub_nt.append(nt)
        tmax = np.zeros(nt)
        for c in range(N_CORES):
            block = _blockify_s(s, core_subs[c][s], nt)
            d = np.where(
                block >= 0, deg_all[np.clip(c * DPC + block, 0, N_NODES - 1)], 0
            )
            tmax = np.maximum(tmax, d.reshape(nt, 128).max(1))
        for t in range(nt):
            tiles.append((s, _grid_up(tmax[t])))
    n_tiles = len(tiles)

    # runs of equal (sub, D) packed into gather calls of <= WQMAX quad cols
    runs = []
    i = 0
    while i < n_tiles:
        s, D = tiles[i]
        n = 1
        while i + n < n_tiles and tiles[i + n] == (s, D):
            n += 1
        runs.append((s, i, D, n))
        i += n
    # pack runs into calls by SLOT width; a call's quad width is its slot
    # width rounded up to a multiple of 4 (dead pad slots at the call end)
    calls = []
    cur_s, cur, cw = None, [], 0
    for (s, t0, D, n) in runs:
        rem_t0, rem_n = t0, n
        while rem_n:
            lim = 32 if len(calls) < 2 else SMAX  # small first calls: ramp
            lim = max(lim, D)
            if cur and (cur_s != s or cw + D > lim):
                calls.append((cur_s, 0, -(-cw // 4), cur))
                cur_s, cur, cw = None, [], 0
                continue
            take = min(rem_n, (lim - cw) // D)
            assert take > 0, (s, D, n, cw, lim)
            cur_s = s
            cur.append((rem_t0, D, take, cw))
            cw += D * take
            rem_t0 += take
            rem_n -= take
    if cur:
        calls.append((cur_s, 0, -(-cw // 4), cur))
    # split a small tail off the last call so the drain is short
    s_l, _, _, parts_l = calls[-1]
    tot_l = sum(D * ntc for (_, D, ntc, _) in parts_l)
    if tot_l > 32:
        target = tot_l - 16
        p1, p2, acc, w1, w2 = [], [], 0, 0, 0
        for (t0, D, ntc, lc) in parts_l:
            for tl in range(ntc):
                if acc < target:
                    if p1 and p1[-1][0] + p1[-1][2] == t0 + tl and p1[-1][1] == D:
                        p1[-1] = (p1[-1][0], D, p1[-1][2] + 1, p1[-1][3])
                    else:
                        p1.append((t0 + tl, D, 1, w1))
                    w1 += D
                else:
                    if p2 and p2[-1][0] + p2[-1][2] == t0 + tl and p2[-1][1] == D:
                        p2[-1] = (p2[-1][0], D, p2[-1][2] + 1, p2[-1][3])
                    else:
                        p2.append((t0 + tl, D, 1, w2))
                    w2 += D
                acc += D
        p1 = [tuple(x) for x in p1]
        p2 = [tuple(x) for x in p2]
        calls[-1] = (s_l, 0, -(-w1 // 4), p1)
        calls.append((s_l, 0, -(-w2 // 4), p2))
    qcol = 0
    for j, (s, _, Wq, parts) in enumerate(calls):
        calls[j] = (s, qcol, Wq, parts)
        qcol += Wq
    wq_total = qcol
    w_total = 4 * wq_total
    tile_col0 = np.zeros(n_tiles, np.int64)
    for (s, qcol0, Wq, parts) in calls:
        for (t0, D, ntc, lc) in parts:
            for tl in range(ntc):
                tile_col0[t0 + tl] = 4 * qcol0 + lc + tl * D
    sub_of_tile = np.array([s for (s, D) in tiles], np.int64)

    perms, idx16s, qrowss, edgess = [], [], [], []
    rsub_max = 0
    percore = []
    for c in range(N_CORES):
        perm = np.full(n_tiles * 128, -1, dtype=np.int64)
        ti = 0
        for s in range(NSUB):
            nt = sub_nt[s]
            perm[ti * 128 : (ti + nt) * 128] = _blockify_s(s, core_subs[c][s], nt)
            ti += nt

        real = perm >= 0
        pos_of_dst = np.empty(DPC, np.int64)
        pos_of_dst[perm[real]] = np.flatnonzero(real)
        gtile_of_dst = pos_of_dst // 128
        p_of_dst = pos_of_dst % 128

        lo, hi = starts_all[c * DPC], starts_all[(c + 1) * DPC]
        e_src = src_s[lo:hi].astype(np.int32)
        e_dstl = dst_s[lo:hi] - c * DPC
        e_k = k_s[lo:hi]
        e_tile = gtile_of_dst[e_dstl]
        e_p = p_of_dst[e_dstl].astype(np.int32)
        e_col = (tile_col0[e_tile] + e_k).astype(np.int32)

        # slot-level source matrix (-1 = pad), then quads + per-sub dedup
        S = np.full((128, w_total), -1, np.int32)
        S[e_p, e_col] = e_src
        idxq = np.zeros((128, wq_total), np.int32)
        qrows_l, qnodes_l = [], []
        for s in range(NSUB):
            qsel = [
                (qcol0, Wq)
                for (ss, qcol0, Wq, parts) in calls
                if ss == s
            ]
            cols = np.concatenate(
                [np.arange(q0, q0 + Wq) for (q0, Wq) in qsel]
            )
            quads = S[:, (4 * cols[:, None] + np.arange(4)).reshape(-1)]
            quads = quads.reshape(128, len(cols), 4)
            flat = np.ascontiguousarray(quads.reshape(-1, 4))
            u, inv = np.unique(flat.view("V16").ravel(), return_inverse=True)
            nu = len(u)
            rsub_max = max(rsub_max, nu)
            uq = u.view(np.int32).reshape(-1, 4)
            idxq[:, cols] = inv.reshape(128, len(cols))
            qrows_l.append(uq)
        percore.append((perm, idxq, qrows_l, (e_p, e_col, e_src)))

    rsubq = -(-int(rsub_max) // 128) * 128
    for c in range(N_CORES):
        perm, idxq, qrows_l, edges = percore[c]
        idx16 = np.zeros((128, 8 * wq_total), np.int16)
        for (s, qcol0, Wq, parts) in calls:
            flat = idxq[:, qcol0 : qcol0 + Wq].T.ravel()
            idx16[:, 8 * qcol0 : 8 * (qcol0 + Wq)] = np.tile(
                flat.reshape(-1, 16).T, (8, 1)
            ).astype(np.int16)
        rows = np.concatenate(
            [s * rsubq + np.arange(len(qrows_l[s])) for s in range(NSUB)]
        )
        nodes = np.concatenate(qrows_l, axis=0)
        perms.append(perm)
        idx16s.append(idx16)
        qrowss.append((rows, nodes))
        edgess.append(edges)

    return Schedule(
        n_tiles,
        w_total,
        wq_total,
        rsubq,
        tiles,
        calls,
        perms,
        idx16s,
        qrowss,
        edgess,
    )


# ---------------------------------------------------------------- prog A
def build_progA(n_loc=DPC, in_c=IN_C, f=F):
    nc = bacc.Bacc("TRN2", target_bir_lowering=False, debug=False, num_devices=N_CORES)
    xT = nc.dram_tensor("xT", [in_c, n_loc], F16, kind="ExternalInput").ap()
    W = nc.dram_tensor("W", [in_c, f], F16, kind="ExternalInput").ap()
    bW = nc.dram_tensor("bW", [f, 1], F32, kind="ExternalInput").ap()
    WA = nc.dram_tensor("WA", [in_c, 2], F16, kind="ExternalInput").ap()
    bA2 = nc.dram_tensor("bA2", [2, 1], F32, kind="ExternalInput").ap()
    whT = nc.dram_tensor("whT", [f, n_loc], F32, kind="ExternalOutput").ap()
    s = nc.dram_tensor("s", [2, n_loc], F32, kind="ExternalOutput").ap()

    with tile.TileContext(nc) as tc:
        with tc.tile_pool(name="sb", bufs=1) as pool, tc.tile_pool(
            name="ps", bufs=4, space="PSUM"
        ) as pps, tc.tile_pool(name="sb2", bufs=3) as pool2:
            W_sb = pool.tile([in_c, f], F16)
            nc.sync.dma_start(out=W_sb[:], in_=W[:, :])
            bW_sb = pool.tile([f, 1], F32)
            nc.sync.dma_start(out=bW_sb[:], in_=bW[:, :])
            WA_sb = pool.tile([in_c, 2], F16)
            nc.sync.dma_start(out=WA_sb[:], in_=WA[:, :])
            bA2_sb = pool.tile([2, 1], F32)
            nc.sync.dma_start(out=bA2_sb[:], in_=bA2[:, :])
            xT_sb = pool.tile([in_c, n_loc], F16)
            XCH = 3125
            for x0 in range(0, n_loc, XCH):
                xc = min(XCH, n_loc - x0)
                nc.sync.dma_start(
                    out=xT_sb[:, x0 : x0 + xc], in_=xT[:, x0 : x0 + xc]
                )

            CH = 512
            GRP = 4  # store in 2048-column groups
            wh_g = None
            s_g = None
            for c0 in range(0, n_loc, CH):
                ch = min(CH, n_loc - c0)
                gi = (c0 // CH) % GRP
                if gi == 0:
                    wh_g = pool2.tile([f, GRP * CH], F32, tag="whg")
                    s_g = pool2.tile([2, GRP * CH], F32, tag="sg")
                ps_w = pps.tile([f, CH], F32, space="PSUM")
                nc.tensor.matmul(
                    out=ps_w[:, :ch],
                    lhsT=W_sb[:],
                    rhs=xT_sb[:, c0 : c0 + ch],
                    start=True,
                    stop=True,
                )
                nc.scalar.activation(
                    out=wh_g[:, gi * CH : gi * CH + ch],
                    in_=ps_w[:, :ch],
                    func=mybir.ActivationFunctionType.Identity,
                    bias=bW_sb[:],
                )
                ps_s = pps.tile([2, CH], F32, space="PSUM")
                nc.tensor.matmul(
                    out=ps_s[:, :ch],
                    lhsT=WA_sb[:],
                    rhs=xT_sb[:, c0 : c0 + ch],
                    start=True,
                    stop=True,
                )
                nc.vector.tensor_scalar(
                    out=s_g[:, gi * CH : gi * CH + ch],
                    in0=ps_s[:, :ch],
                    scalar1=bA2_sb[:, 0:1],
                    scalar2=None,
                    op0=mybir.AluOpType.add,
                )
                if gi == GRP - 1 or c0 + ch >= n_loc:
                    g0 = (c0 // CH // GRP) * GRP * CH
                    gl = c0 + ch - g0
                    nc.sync.dma_start(
                        out=whT[:, g0 : g0 + gl], in_=wh_g[:, :gl]
                    )
                    nc.sync.dma_start(out=s[:, g0 : g0 + gl], in_=s_g[:, :gl])
    nc.compile()
    return nc


# ---------------------------------------------------------------- prog B
def build_progB(sched: Schedule, f=F):
    NT = sched.n_tiles
    WTOT = sched.w_total
    WQTOT = sched.wq_total
    RSUBQ = sched.rsubq
    nc = bacc.Bacc("TRN2", target_bir_lowering=False, debug=False, num_devices=N_CORES)
    tableq = nc.dram_tensor(
        "tableq", [NSUB * RSUBQ, 2 * f], F32, kind="ExternalInput"
    ).ap()
    idx_d = nc.dram_tensor("idx", [128, 8 * WQTOT], I16, kind="ExternalInput").ap()
    sj_d = nc.dram_tensor("sj", [128, WTOT], F32, kind="ExternalInput").ap()
    si_d = nc.dram_tensor("si", [128, NT], F32, kind="ExternalInput").ap()
    # packed small consts: cols 0-63 Wn, 64 bWn, 65-66 As
    wp_d = nc.dram_tensor("wpack", [f, f + 4], F32, kind="ExternalInput").ap()
    whnT = nc.dram_tensor("whnT", [f, NT * 128], F32, kind="ExternalOutput").ap()
    sn = nc.dram_tensor("sn", [2, NT * 128], F32, kind="ExternalOutput").ap()

    X = mybir.AxisListType.X
    AF = mybir.ActivationFunctionType
    OP = mybir.AluOpType

    def v(ap, dims, off=0):
        return dataclasses.replace(
            ap,
            ap=[list(ap.ap[0])] + [list(d) for d in dims],
            offset=ap.offset + off,
        )

    nq = min(4, nc.num_swdge_queues)
    NTCMAX = max(ntc for (_, _, _, parts) in sched.calls for (_, _, ntc, _) in parts)

    with tile.TileContext(nc) as tc:
        with tc.tile_pool(name="const", bufs=1) as pc, tc.tile_pool(
            name="gat", bufs=3
        ) as pg, tc.tile_pool(name="exw", bufs=3) as px, tc.tile_pool(
            name="work", bufs=3
        ) as pw, tc.tile_pool(name="ht", bufs=2) as ph, tc.tile_pool(
            name="ps", bufs=2, space="PSUM"
        ) as pps, tc.tile_pool(name="ep", bufs=3) as pep:
            si_sb = pc.tile([128, NT], F32)
            nc.sync.dma_start(out=si_sb[:], in_=si_d[:, :])
            sj_sb = pc.tile([128, WTOT], F32)
            idx_sb = pc.tile([128, 8 * WQTOT], I16)
            wp_sb = pc.tile([f, f + 4], F32)
            Wn_sb = wp_sb[:, :f]
            bWn_sb = wp_sb[:, f : f + 1]
            As_sb = wp_sb[:, f + 1 : f + 3]
            ident = pc.tile([128, 128], F16)

            def emit_consts():
                # deferred past the first call's gather so the startup HWDGE
                # FIFO isn't serialized ahead of it
                nc.sync.dma_start(out=wp_sb[:], in_=wp_d[:, :])
                make_identity(nc, ident[:])

            ps_ch = None

            def flush_chunk(ck, ntl):
                cols = ntl * 128
                hTL = pep.tile([f, CHT * 128], F32, tag="hTL")
                nc.scalar.activation(
                    out=hTL[:, :cols],
                    in_=ps_ch[:, :cols],
                    func=AF.Identity,
                )
                ps_w = pps.tile([f, CHT * 128], F32, tag="psw", space="PSUM")
                nc.tensor.matmul(
                    out=ps_w[:, :cols],
                    lhsT=Wn_sb[:],
                    rhs=hTL[:, :cols],
                    start=True,
                    stop=True,
                )
                whn_sb = pep.tile([f, CHT * 128], F32, tag="whn")
                nc.scalar.activation(
                    out=whn_sb[:, :cols],
                    in_=ps_w[:, :cols],
                    func=AF.Identity,
                    bias=bWn_sb[:],
                )
                nc.sync.dma_start(
                    out=whnT[:, ck * CHT * 128 : ck * CHT * 128 + cols],
                    in_=whn_sb[:, :cols],
                )
                ps_s = pps.tile([2, CHT * 128], F32, tag="pss", space="PSUM")
                nc.tensor.matmul(
                    out=ps_s[:, :cols],
                    lhsT=As_sb,
                    rhs=whn_sb[:, :cols],
                    start=True,
                    stop=True,
                )
                s_sb = pep.tile([2, CHT * 128], F32, tag="ssb")
                nc.scalar.activation(
                    out=s_sb[:, :cols], in_=ps_s[:, :cols], func=AF.Identity
                )
                nc.sync.dma_start(
                    out=sn[:, ck * CHT * 128 : ck * CHT * 128 + cols],
                    in_=s_sb[:, :cols],
                )

            gq = 0

            def stage1(ci):
                nonlocal gq
                s, qcol0, Wq, parts = sched.calls[ci]
                # per-call slices of the idx / sj constants (shorter ramp);
                # sj first: the DVE's epre only needs sj+si, not the gather
                nc.sync.dma_start(
                    out=sj_sb[:, 4 * qcol0 : 4 * (qcol0 + Wq)],
                    in_=sj_d[:, 4 * qcol0 : 4 * (qcol0 + Wq)],
                )
                nc.sync.dma_start(
                    out=idx_sb[:, 8 * qcol0 : 8 * (qcol0 + Wq)],
                    in_=idx_d[:, 8 * qcol0 : 8 * (qcol0 + Wq)],
                )
                gbuf = pg.tile([128, WQMAX * 2 * f], F32, tag="gbuf")
                # hw limit: <=1024 indices per dma_gather -> <=8 quad columns
                for j0 in range(0, Wq, 8):
                    jw = min(8, Wq - j0)
                    nc.gpsimd.dma_gather(
                        out_ap=v(
                            gbuf[:], [(2 * f, jw), (1, 2 * f)], off=j0 * 2 * f
                        ),
                        in_ap=tableq[s * RSUBQ : (s + 1) * RSUBQ, :],
                        idxs_ap=idx_sb[:, 8 * (qcol0 + j0) : 8 * (qcol0 + j0 + jw)],
                        num_idxs=128 * jw,
                        num_idxs_reg=128 * jw,
                        elem_size=2 * f,
                        queue_num=gq % nq,
                    )
                    gq += 1
                gbuf16 = gbuf[:].bitcast(F16)  # slot i feats at f16 cols [64i,+64)
                exw = px.tile([128, WQMAX * 4 * f], F16, tag="exw")

                # attention logits + exp for every part
                for (t0, D, ntc, lc) in parts:
                    Wr = D * ntc
                    ls = lc  # slot offset within call
                    sc = 4 * qcol0 + lc  # global slot column
                    # e_pre = sj + si'  (si' = si + bA; sj = NEG_BIG at pads)
                    epre = pw.tile([128, 4 * WQMAX], F32, tag="epre")
                    nc.vector.tensor_tensor(
                        out=v(epre[:], [(D, ntc), (1, D)]),
                        in0=v(sj_sb[:], [(D, ntc), (1, D)], off=sc),
                        in1=si_sb[:, t0 : t0 + ntc].to_broadcast([128, ntc, D]),
                        op=OP.add,
                    )
                    # e = leaky_relu(e_pre) on Act (parametric_relu shares
                    # the exp activation table; alpha is the slope)
                    e1 = pw.tile([128, 4 * WQMAX], F32, tag="e1")
                    nc.scalar.activation(
                        out=e1[:, :Wr],
                        in_=epre[:, :Wr],
                        func=AF.Prelu,
                        alpha=ALPHA,
                    )
                    # exp without max-subtraction: the host folds a global
                    # shift into si so logits stay < 8 (exp < 3000, well
                    # inside fp16); pad slots (sj = -60) underflow to ~1e-5
                    # so denominators stay finite without an eps op
                    # exp, broadcast-expanded across the feature axis (fp16)
                    nc.scalar.activation(
                        out=v(exw[:], [(f * D, ntc), (f, D), (1, f)], off=ls * f),
                        in_=v(e1[:], [(D, ntc), (1, D), (0, f)]),
                        func=AF.Exp,
                    )
                return gbuf16, exw

            def stage2(ci, gbuf16, exw):
                nonlocal ps_ch
                s, qcol0, Wq, parts = sched.calls[ci]
                # denominator, weighted message sum, epilogue
                for (t0, D, ntc, lc) in parts:
                    Wr = D * ntc
                    ls = lc
                    den = pw.tile([128, NTCMAX], F32, tag="den")
                    nc.vector.tensor_reduce(
                        out=den[:, :ntc],
                        in_=v(exw[:], [(f * D, ntc), (f, D)], off=ls * f),
                        axis=X,
                        op=OP.add,
                    )
                    rden = pw.tile([128, NTCMAX], F32, tag="rden")
                    nc.vector.reciprocal(out=rden[:, :ntc], in_=den[:, :ntc])
                    # weighted messages: exw *= Wh16 (in place, 2x fp16)
                    nc.vector.tensor_tensor(
                        out=v(exw[:], [(f * D, ntc), (f, D), (1, f)], off=ls * f),
                        in0=v(gbuf16, [(f * D, ntc), (f, D), (1, f)], off=ls * f),
                        in1=v(exw[:], [(f * D, ntc), (f, D), (1, f)], off=ls * f),
                        op=OP.mult,
                    )
                    # halving-tree sum over slots -> slot 0 of each tile
                    cur = D
                    while cur > 1:
                        if cur % 2:
                            nc.vector.tensor_tensor(
                                out=v(exw[:], [(f * D, ntc), (1, f)], off=ls * f),
                                in0=v(exw[:], [(f * D, ntc), (1, f)], off=ls * f),
                                in1=v(
                                    exw[:],
                                    [(f * D, ntc), (1, f)],
                                    off=(ls + cur - 1) * f,
                                ),
                                op=OP.add,
                            )
                            cur -= 1
                        h = cur // 2
                        nc.vector.tensor_tensor(
                            out=v(exw[:], [(f * D, ntc), (f, h), (1, f)], off=ls * f),
                            in0=v(exw[:], [(f * D, ntc), (f, h), (1, f)], off=ls * f),
                            in1=v(
                                exw[:],
                                [(f * D, ntc), (f, h), (1, f)],
                                off=(ls + h) * f,
                            ),
                            op=OP.add,
                        )
                        cur = h
                    # normalize + leaky (dst-major, fp16), then transpose
                    ht = ph.tile([128, NTCMAX * f], F16, tag="ht")
                    nc.vector.tensor_tensor(
                        out=v(ht[:], [(f, ntc), (1, f)]),
                        in0=v(exw[:], [(f * D, ntc), (1, f)], off=ls * f),
                        in1=rden[:, :ntc].to_broadcast([128, ntc, f]),
                        op=OP.mult,
                    )
                    ht2 = ph.tile([128, NTCMAX * f], F16, tag="ht2")
                    nc.vector.tensor_scalar(
                        out=ht2[:, : ntc * f],
                        in0=ht[:, : ntc * f],
                        scalar1=ALPHA,
                        scalar2=None,
                        op0=OP.mult,
                    )
                    nc.vector.tensor_tensor(
                        out=ht[:, : ntc * f],
                        in0=ht[:, : ntc * f],
                        in1=ht2[:, : ntc * f],
                        op=OP.max,
                    )
                    for tl in range(ntc):
                        t = t0 + tl
                        j = t % CHT
                        if j == 0:
                            ps_ch = pps.tile(
                                [f, CHT * 128], F16, tag="psch", space="PSUM"
                            )
                        nc.tensor.transpose(
                            out=ps_ch[:, j * 128 : (j + 1) * 128],
                            in_=ht[:, tl * f : (tl + 1) * f],
                            identity=ident[:],
                        )
                        if j == CHT - 1 or t == NT - 1:
                            flush_chunk(t // CHT, j + 1)

            # software pipeline: stage1 of call N+1 is emitted before stage2
            # of call N so the DVE never stalls on Act's exp at call
            # boundaries
            prev = None
            for ci in range(len(sched.calls)):
                ctx = stage1(ci)
                if ci == 0:
                    emit_consts()
                if prev is not None:
                    stage2(prev[0], *prev[1])
                prev = (ci, ctx)
            stage2(prev[0], *prev[1])
    nc.compile()
    return nc


# ---------------------------------------------------------------- driver
_cache = {}
TRACE = False
LAST_HW_NS = []
LAST_RESULTS = []


def _run(nc, in_maps, cores):
    res = bass_utils.run_bass_kernel_spmd(nc, in_maps, core_ids=cores, trace=TRACE)
    if TRACE:
        LAST_RESULTS.append(res)
        if res.exec_time_ns:
            LAST_HW_NS.append(res.exec_time_ns)
    return res


def _get_schedule(edge_index):
    fp = hashlib.sha1(np.ascontiguousarray(edge_index)).hexdigest()
    key = ("sched", fp)
    if key not in _cache:
        _cache[key] = build_schedule(edge_index)
    return _cache[key]


def _pack_table(sched, c, wh16, sj_full):
    rows, nodes = sched.qrows[c]
    table = np.zeros((NSUB * sched.rsubq, 2 * F), np.float32)
    blk = wh16[np.clip(nodes, 0, N_NODES - 1)]
    blk[nodes < 0] = 0
    table[rows] = blk.reshape(len(rows), 4 * F).view(np.float32)
    return table


def kernel(x, edge_index, W1, bW1, A1, bA1, W2, bW2, A2, bA2, Wfc, bfc):
    x = np.asarray(x, dtype=np.float32)
    edge_index = np.asarray(edge_index)
    W1 = np.asarray(W1, np.float32)
    bW1 = np.asarray(bW1, np.float32)
    A1 = np.asarray(A1, np.float32)
    bA1 = np.asarray(bA1, np.float32)
    W2 = np.asarray(W2, np.float32)
    bW2 = np.asarray(bW2, np.float32)
    A2 = np.asarray(A2, np.float32)
    bA2 = np.asarray(bA2, np.float32)
    Wfc = np.asarray(Wfc, np.float32)
    bfc = np.asarray(bfc, np.float32)

    sched = _get_schedule(edge_index)
    cores = list(range(N_CORES))

    if "A" not in _cache:
        _cache["A"] = build_progA()
    ncA = _cache["A"]
    inA = []
    x16 = x.astype(np.float16)
    A1cat = np.concatenate([A1[:F], A1[F:]], axis=1)  # [64, 2]
    WA = (W1 @ A1cat).astype(np.float16)  # [128, 2]
    bA2v = (bW1 @ A1cat).reshape(2, 1).astype(np.float32)
    for c in cores:
        xT = np.ascontiguousarray(x16[c * DPC : (c + 1) * DPC].T)
        inA.append(
            {
                "xT": xT,
                "W": W1.astype(np.float16),
                "bW": bW1.reshape(F, 1),
                "WA": WA,
                "bA2": bA2v,
            }
        )
    LAST_HW_NS.clear()
    LAST_RESULTS.clear()
    resA = _run(ncA, inA, cores)
    wh = np.concatenate([resA.results[c]["whT"].T for c in cores], axis=0)
    s_all = np.concatenate([resA.results[c]["s"] for c in cores], axis=1)
    si_full, sj_full = s_all[0], s_all[1]

    key = ("B", sched.n_tiles, sched.wq_total, sched.rsubq, tuple(sched.tiles))
    if key not in _cache:
        _cache[key] = build_progB(sched)
    ncB = _cache[key]

    def launch_B(wh_full, si_f, sj_f, bA, Wn, bWn, An):
        bA0 = np.float32(bA.reshape(-1)[0])
        wh16 = wh_full.astype(np.float16)
        wpack = np.zeros((F, F + 4), np.float32)
        wpack[:, :F] = Wn
        wpack[:, F] = bWn.reshape(F)
        wpack[:, F + 1 : F + 3] = An
        inB = []
        shift = np.float32(max(0.0, float(si_f.max() + sj_f.max() + bA0) - 8.0))
        for c in cores:
            perm = sched.perms[c]
            real = perm >= 0
            gids = c * DPC + perm[real]
            tmp = np.zeros(sched.n_tiles * 128, np.float32)
            tmp[real] = si_f[gids] + bA0 - shift
            si_arr = np.ascontiguousarray(tmp.reshape(sched.n_tiles, 128).T)
            e_p, e_col, e_src = sched.edges[c]
            sj_arr = np.full((128, sched.w_total), np.float32(PAD_SJ))
            sj_arr[e_p, e_col] = sj_f[e_src]
            inB.append(
                {
                    "tableq": _pack_table(sched, c, wh16, sj_f),
                    "idx": sched.idx16[c],
                    "sj": sj_arr,
                    "si": si_arr,
                    "wpack": wpack,
                }
            )
        res = _run(ncB, inB, cores)
        whn = np.zeros((N_NODES, F), np.float32)
        sn_i = np.zeros(N_NODES, np.float32)
        sn_j = np.zeros(N_NODES, np.float32)
        for c in cores:
            perm = sched.perms[c]
            real = perm >= 0
            gids = c * DPC + perm[real]
            whn[gids] = res.results[c]["whnT"].T[real]
            sn_c = res.results[c]["sn"]
            sn_i[gids] = sn_c[0][real]
            sn_j[gids] = sn_c[1][real]
        return whn, sn_i, sn_j

    As2 = np.ascontiguousarray(np.concatenate([A2[:F], A2[F:]], axis=1))
    wh2, si2, sj2 = launch_B(wh, si_full, sj_full, bA1, W2, bW2, As2)
    out, _, _ = launch_B(wh2, si2, sj2, bA2, Wfc, bfc, np.zeros((F, 2), np.float32))
    return out.astype(np.float32)



# revision 24
# speedup vs baseline: 1.2919x; 1.2919x over previous
"""GAT (2-layer) on 8 NeuronCores — Bass/Tile kernel.

Strategy (dst-sharded graph parallel, streamed tables):
  - Each core owns 12500 destination nodes, degree-sorted descending into
    98 tiles of 128 dsts. Per-tile slot capacity D = the max degree of the
    tile across all cores (exact, no grid rounding — degrees span 2..36 so
    pad waste is ~1-3%).
  - The halo "gather" is a plain 2D stream: the host packs each core's
    table in exact slot-consumption order ([128 partitions, w_total x 64
    fp16] = one 128B row per slot, zeros at pads), so the kernel issues
    one contiguous dma_start per call instead of dma_gather — no index
    tables, no SWDGE descriptor generation, Pool is freed for compute.
  - Launch A: per-core Wh1^T = (x W1 + b)^T (fp16 out) + attention
    scalars s_i/s_j.
  - Host between launches: packs the slot tables from device-computed Wh
    (fp16 row gather = pure layout), expands s_j / (s_i + bA - shift) to
    slot level (softmax is shift-invariant; keeps exp inside fp16 range),
    and prepares the fused epilogue weights [Wn | Wn@As] (the same
    weight-folding progA uses). Index/layout work only; all value FLOPs
    run on device.
  - Launch B (x2, one per GAT layer): per call (<=128 slot cols):
    stream table + [sj|si] slices, epre = sj+si (DVE), e1 = leaky (Act),
    then per part a static planner balances three engines: exp is either
    broadcast-expanded to feature width on Act with the 2x-rate fp16
    multiply on DVE, or kept per-slot with a fused broadcast multiply on
    Pool (gpsimd). The slot halving-tree sum is likewise split between
    DVE and Pool. Segment softmax denominators reduce on DVE; normalize
    on DVE; leaky of the layer output rides the PSUM->SBUF Act copy; the
    epilogue is one fused matmul [Wn | Wn@As] producing whn and the next
    layer's attention scalars in a single fp16 output tensor.
"""

import dataclasses
import hashlib
import numpy as np

import concourse.bacc as bacc
import concourse.tile as tile
from concourse import bass, mybir, bass_utils
from concourse.masks import make_identity

F32 = mybir.dt.float32
F16 = mybir.dt.float16

N_NODES = 100000
N_CORES = 8
DPC = N_NODES // N_CORES
F = 64
IN_C = 128
SMAX = 128  # slot columns per call
EXPW = 32  # exp expansion width; DVE multiplies in F/EXPW passes
CHT = 4  # tiles per epilogue chunk (512 dsts)
PAD_SJ = -60.0
ALPHA = 0.2

# planner cost constants (ns); see TRN2Spec in concourse/hw_specs.py
ACT_EL = 0.833
DVE2X = 0.5208
DVE1X = 1.0417
POOL_EL = 1.984  # 0.833 / 0.42 gpsimd Add/Multiply efficiency
FIX_ACT = 185.0
FIX_DVE = 146.0
FIX_POOL = 60.0


@dataclasses.dataclass
class Schedule:
    n_tiles: int
    w_total: int  # slot columns
    tiles: list  # per tile: D
    calls: list  # (col0, W, parts) ; parts: [(t0, D, ntc, lc, m_pool, t_pool)]
    perms: list  # per core: int64 [n_tiles*128], local dst or -1
    slot_srcs: list  # per core: int32 [128, w_total], global src or -1
    edges: list  # per core: (e_p, e_col, e_src, e_dstg) for sj/si expansion


def build_schedule(edge_index: np.ndarray) -> Schedule:
    src = np.asarray(edge_index[0], dtype=np.int64)
    dst = np.asarray(edge_index[1], dtype=np.int64)
    E = src.shape[0]
    order = np.argsort(dst, kind="stable")
    src_s = src[order]
    dst_s = dst[order]
    deg_all = np.bincount(dst, minlength=N_NODES)
    starts_all = np.concatenate([[0], np.cumsum(deg_all)])
    k_s = np.arange(E) - starts_all[dst_s]

    nt = -(-DPC // 128)
    # degree-descending per core; shared tile capacity = max over cores
    perms = []
    tmax = np.zeros(nt)
    for c in range(N_CORES):
        deg = deg_all[c * DPC : (c + 1) * DPC]
        rank = np.argsort(deg, kind="stable")[::-1]
        perm = np.full(nt * 128, -1, dtype=np.int64)
        perm[:DPC] = rank
        perms.append(perm)
        d = np.where(perm >= 0, deg[np.clip(perm, 0, DPC - 1)], 0)
        tmax = np.maximum(tmax, d.reshape(nt, 128).max(1))
    tiles = [int(x) for x in tmax]
    n_tiles = nt

    # pack runs of equal D into calls of <= SMAX slot columns (ramped)
    runs = []
    i = 0
    while i < n_tiles:
        D = tiles[i]
        n = 1
        while i + n < n_tiles and tiles[i + n] == D:
            n += 1
        runs.append((i, D, n))
        i += n
    calls = []  # (col0, W, parts)
    cur, cw = [], 0
    for (t0, D, n) in runs:
        rem_t0, rem_n = t0, n
        while rem_n:
            lim = (32, 64)[len(calls)] if len(calls) < 2 else SMAX
            lim = max(lim, D)
            if cur and cw + D > lim:
                calls.append((0, cw, cur))
                cur, cw = [], 0
                continue
            take = min(rem_n, (lim - cw) // D)
            cur.append((rem_t0, D, take, cw))
            cw += D * take
            rem_t0 += take
            rem_n -= take
    if cur:
        calls.append((0, cw, cur))
    # split a small tail off the last call so the drain is short
    _, wl, parts_l = calls[-1]
    if wl > 32:
        target = wl - 16
        p1, p2, acc, w1, w2 = [], [], 0, 0, 0
        for (t0, D, ntc, lc) in parts_l:
            for tl in range(ntc):
                dstp, w = (p1, w1) if acc < target else (p2, w2)
                if dstp and dstp[-1][0] + dstp[-1][2] == t0 + tl and dstp[-1][1] == D:
                    dstp[-1] = (dstp[-1][0], D, dstp[-1][2] + 1, dstp[-1][3])
                else:
                    dstp.append((t0 + tl, D, 1, w))
                if acc < target:
                    w1 += D
                else:
                    w2 += D
                acc += D
        calls[-1] = (0, w1, [tuple(x) for x in p1])
        calls.append((0, w2, [tuple(x) for x in p2]))
    col = 0
    for j, (_, W, parts) in enumerate(calls):
        calls[j] = (col, W, parts)
        col += W
    w_total = col

    # chop parts into <=3-tile units so the engine planner can split work
    # within single-part calls
    chopped = []
    for (col0, W, parts) in calls:
        np_ = []
        for (t0, D, ntc, lc) in parts:
            o = 0
            while o < ntc:
                k = min(3, ntc - o)
                np_.append((t0 + o, D, k, lc + o * D))
                o += k
        chopped.append((col0, W, np_))
    calls = chopped

    # --- static engine planner: balance Act / DVE / Pool per call ---------
    # (local balance — global-only balancing lets consecutive parts pile on
    # one engine and the in-order engine streams seesaw)
    planned_calls = []
    flush_per_tile = 2 * (FIX_ACT + 512 * ACT_EL) / CHT
    for (col0, W, parts) in calls:
        ntc_call = sum(p[2] for p in parts)
        actT = FIX_ACT + W * ACT_EL + ntc_call * flush_per_tile  # e1 + flush
        dveT = 2 * FIX_DVE + W * DVE1X  # epre + reciprocal
        poolT = 0.0
        newparts = []
        for (t0, D, ntc, lc) in parts:
            S = D * ntc
            Eel = 64.0 * S
            Tel = 64.0 * (S - ntc)
            lev = max(1, int(np.ceil(np.log2(max(D, 2)))))
            dveT += FIX_DVE + S * DVE1X  # den reduce
            dveT += FIX_DVE + ntc * 64 * DVE1X  # normalize
            best = None
            for m_pool in (0, 1):
                for t_pool in (0, 1):
                    a, d, p = actT, dveT, poolT
                    if m_pool:
                        a += FIX_ACT + S * ACT_EL
                        p += FIX_POOL + Eel * POOL_EL
                    else:
                        a += FIX_ACT + EXPW * S * ACT_EL
                        d += (64 // EXPW) * FIX_DVE + Eel * DVE2X
                    if t_pool:
                        p += lev * FIX_POOL + Tel * POOL_EL
                    else:
                        d += lev * FIX_DVE + Tel * DVE2X
                    mk = max(a, d, p)
                    if best is None or mk < best[0]:
                        best = (mk, m_pool, t_pool, a, d, p)
            _, m_pool, t_pool, actT, dveT, poolT = best
            newparts.append((t0, D, ntc, lc, m_pool, t_pool))
        planned_calls.append((col0, W, newparts))
    calls = planned_calls

    # --- per-core slot-level source map ----------------------------------
    tile_col0 = np.zeros(n_tiles, np.int64)
    for (col0, W, parts) in calls:
        for (t0, D, ntc, lc, _, _) in parts:
            for tl in range(ntc):
                tile_col0[t0 + tl] = col0 + lc + tl * D

    slot_srcs, edges = [], []
    for c in range(N_CORES):
        perm = perms[c]
        real = perm >= 0
        pos_of_dst = np.empty(DPC, np.int64)
        pos_of_dst[perm[real]] = np.flatnonzero(real)
        gtile_of_dst = pos_of_dst // 128
        p_of_dst = pos_of_dst % 128

        lo, hi = starts_all[c * DPC], starts_all[(c + 1) * DPC]
        e_src = src_s[lo:hi].astype(np.int32)
        e_dstl = dst_s[lo:hi] - c * DPC
        e_k = k_s[lo:hi]
        e_tile = gtile_of_dst[e_dstl]
        e_p = p_of_dst[e_dstl].astype(np.int32)
        e_col = (tile_col0[e_tile] + e_k).astype(np.int32)
        S = np.full((128, w_total), -1, np.int32)
        S[e_p, e_col] = e_src
        slot_srcs.append(S)
        edges.append((e_p, e_col, e_src, (c * DPC + e_dstl).astype(np.int32)))

    return Schedule(n_tiles, w_total, tiles, calls, perms, slot_srcs, edges)


# ---------------------------------------------------------------- prog A
def build_progA(n_loc=DPC, in_c=IN_C, f=F):
    nc = bacc.Bacc("TRN2", target_bir_lowering=False, debug=False, num_devices=N_CORES)
    xT = nc.dram_tensor("xT", [in_c, n_loc], F16, kind="ExternalInput").ap()
    W = nc.dram_tensor("W", [in_c, f], F16, kind="ExternalInput").ap()
    bW = nc.dram_tensor("bW", [f, 1], F32, kind="ExternalInput").ap()
    WA = nc.dram_tensor("WA", [in_c, 2], F16, kind="ExternalInput").ap()
    bA2 = nc.dram_tensor("bA2", [2, 1], F32, kind="ExternalInput").ap()
    whT = nc.dram_tensor("whT", [f, n_loc], F16, kind="ExternalOutput").ap()
    s = nc.dram_tensor("s", [2, n_loc], F32, kind="ExternalOutput").ap()

    with tile.TileContext(nc) as tc:
        with tc.tile_pool(name="sb", bufs=1) as pool, tc.tile_pool(
            name="ps", bufs=4, space="PSUM"
        ) as pps, tc.tile_pool(name="sb2", bufs=3) as pool2:
            W_sb = pool.tile([in_c, f], F16)
            nc.sync.dma_start(out=W_sb[:], in_=W[:, :])
            bW_sb = pool.tile([f, 1], F32)
            nc.sync.dma_start(out=bW_sb[:], in_=bW[:, :])
            WA_sb = pool.tile([in_c, 2], F16)
            nc.sync.dma_start(out=WA_sb[:], in_=WA[:, :])
            bA2_sb = pool.tile([2, 1], F32)
            nc.sync.dma_start(out=bA2_sb[:], in_=bA2[:, :])
            xT_sb = pool.tile([in_c, n_loc], F16)
            XCH = 3125
            for x0 in range(0, n_loc, XCH):
                xc = min(XCH, n_loc - x0)
                nc.sync.dma_start(
                    out=xT_sb[:, x0 : x0 + xc], in_=xT[:, x0 : x0 + xc]
                )

            CH = 512
            GRP = 4  # store in 2048-column groups
            wh_g = None
            s_g = None
            for c0 in range(0, n_loc, CH):
                ch = min(CH, n_loc - c0)
                gi = (c0 // CH) % GRP
                if gi == 0:
                    wh_g = pool2.tile([f, GRP * CH], F16, tag="whg")
                    s_g = pool2.tile([2, GRP * CH], F32, tag="sg")
                ps_w = pps.tile([f, CH], F32, space="PSUM")
                nc.tensor.matmul(
                    out=ps_w[:, :ch],
                    lhsT=W_sb[:],
                    rhs=xT_sb[:, c0 : c0 + ch],
                    start=True,
                    stop=True,
                )
                nc.scalar.activation(
                    out=wh_g[:, gi * CH : gi * CH + ch],
                    in_=ps_w[:, :ch],
                    func=mybir.ActivationFunctionType.Identity,
                    bias=bW_sb[:],
                )
                ps_s = pps.tile([2, CH], F32, space="PSUM")
                nc.tensor.matmul(
                    out=ps_s[:, :ch],
                    lhsT=WA_sb[:],
                    rhs=xT_sb[:, c0 : c0 + ch],
                    start=True,
                    stop=True,
                )
                nc.vector.tensor_scalar(
                    out=s_g[:, gi * CH : gi * CH + ch],
                    in0=ps_s[:, :ch],
                    scalar1=bA2_sb[:, 0:1],
                    scalar2=None,
                    op0=mybir.AluOpType.add,
                )
                if gi == GRP - 1 or c0 + ch >= n_loc:
                    g0 = (c0 // CH // GRP) * GRP * CH
                    gl = c0 + ch - g0
                    nc.sync.dma_start(
                        out=whT[:, g0 : g0 + gl], in_=wh_g[:, :gl]
                    )
                    nc.sync.dma_start(out=s[:, g0 : g0 + gl], in_=s_g[:, :gl])
    nc.compile()
    return nc


# ---------------------------------------------------------------- prog B
def build_progB(sched: Schedule, f=F):
    NT = sched.n_tiles
    WTOT = sched.w_total
    nc = bacc.Bacc("TRN2", target_bir_lowering=False, debug=False, num_devices=N_CORES)
    tbl = nc.dram_tensor("tbl", [128, WTOT * f], F16, kind="ExternalInput").ap()
    sjsi = nc.dram_tensor("sjsi", [128, 2 * WTOT], F32, kind="ExternalInput").ap()
    # packed: rows 0-63 x cols 0-65 = [Wn | Wn@As]; col 66 = bias (66 rows)
    wp_d = nc.dram_tensor("wpack", [f + 2, f + 3], F32, kind="ExternalInput").ap()
    # [128, 1] replicated -shift, applied as the exp bias (post-leaky, so
    # softmax shift-invariance holds exactly)
    shf_d = nc.dram_tensor("shiftv", [128, 1], F32, kind="ExternalInput").ap()
    comb = nc.dram_tensor("comb", [f + 2, NT * 128], F16, kind="ExternalOutput").ap()

    X = mybir.AxisListType.X
    AF = mybir.ActivationFunctionType
    OP = mybir.AluOpType

    def v(ap, dims, off=0):
        return dataclasses.replace(
            ap,
            ap=[list(ap.ap[0])] + [list(d) for d in dims],
            offset=ap.offset + off,
        )

    NTCMAX = max(
        ntc for (_, _, parts) in sched.calls for (_, _, ntc, _, _, _) in parts
    )
    NTC_CALL = max(
        sum(ntc for (_, _, ntc, _, _, _) in parts) for (_, _, parts) in sched.calls
    )

    with tile.TileContext(nc) as tc:
        with tc.tile_pool(name="const", bufs=1) as pc, tc.tile_pool(
            name="gat", bufs=4
        ) as pg, tc.tile_pool(name="exw", bufs=3) as px, tc.tile_pool(
            name="work", bufs=4
        ) as pw, tc.tile_pool(name="ht", bufs=3) as ph, tc.tile_pool(
            name="ps", bufs=2, space="PSUM"
        ) as pps, tc.tile_pool(name="ep", bufs=3) as pep:
            wp_sb = pc.tile([f + 2, f + 3], F32)
            WC_sb = pc.tile([f, f + 2], F16)
            bias_sb = wp_sb[:, f + 2 : f + 3]
            ident = pc.tile([128, 128], F16)
            shf_sb = pc.tile([128, 1], F32)
            nc.sync.dma_start(out=shf_sb[:], in_=shf_d[:, :])

            def emit_consts():
                # deferred past the first call's stream so the startup HWDGE
                # FIFO isn't serialized ahead of it
                nc.sync.dma_start(out=wp_sb[:], in_=wp_d[:, :])
                nc.vector.tensor_scalar(
                    out=WC_sb[:],
                    in0=wp_sb[: f, : f + 2],
                    scalar1=1.0,
                    scalar2=None,
                    op0=OP.mult,
                )
                make_identity(nc, ident[:])

            ps_ch = None

            def flush_chunk(ck, ntl):
                cols = ntl * 128
                hTL = pep.tile([f, CHT * 128], F16, tag="hTL")
                nc.scalar.activation(
                    out=hTL[:, :cols],
                    in_=ps_ch[:, :cols],
                    func=AF.Prelu,
                    alpha=ALPHA,
                )
                ps_c = pps.tile([f + 2, CHT * 128], F32, tag="psc", space="PSUM")
                nc.tensor.matmul(
                    out=ps_c[:, :cols],
                    lhsT=WC_sb[:],
                    rhs=hTL[:, :cols],
                    start=True,
                    stop=True,
                )
                c_sb = pep.tile([f + 2, CHT * 128], F16, tag="csb")
                nc.scalar.activation(
                    out=c_sb[:, :cols],
                    in_=ps_c[:, :cols],
                    func=AF.Identity,
                    bias=bias_sb,
                )
                nc.sync.dma_start(
                    out=comb[:, ck * CHT * 128 : ck * CHT * 128 + cols],
                    in_=c_sb[:, :cols],
                )

            def stage1(ci):
                col0, W, parts = sched.calls[ci]
                sj_sb = pw.tile([128, 2 * SMAX], F32, tag="sjsi")
                nc.sync.dma_start(
                    out=sj_sb[:, : 2 * W], in_=sjsi[:, 2 * col0 : 2 * col0 + 2 * W]
                )
                gbuf = pg.tile([128, SMAX * f], F16, tag="gbuf")
                nc.sync.dma_start(
                    out=gbuf[:, : W * f], in_=tbl[:, col0 * f : (col0 + W) * f]
                )
                # epre = sj + si' (si' = si + bA - shift; sj = -60 at pads)
                epre = pw.tile([128, SMAX], F32, tag="epre")
                nc.vector.tensor_tensor(
                    out=epre[:, :W],
                    in0=sj_sb[:, :W],
                    in1=sj_sb[:, W : 2 * W],
                    op=OP.add,
                )
                # e = leaky_relu(epre) on Act
                e1 = pw.tile([128, SMAX], F32, tag="e1")
                nc.scalar.activation(
                    out=e1[:, :W], in_=epre[:, :W], func=AF.Prelu, alpha=ALPHA
                )
                exw = None
                eexp = None
                for (t0, D, ntc, lc, m_pool, t_pool) in parts:
                    if m_pool:
                        # per-slot exp only; Pool does the fused multiply
                        if eexp is None:
                            eexp = pw.tile([128, SMAX], F16, tag="eexp")
                        nc.scalar.activation(
                            out=eexp[:, lc : lc + D * ntc],
                            in_=e1[:, lc : lc + D * ntc],
                            func=AF.Exp,
                            bias=shf_sb[:],
                        )
                    else:
                        # exp broadcast-expanded to EXPW on Act; the DVE
                        # multiply re-reads it f/EXPW times at 2x rate
                        if exw is None:
                            exw = px.tile([128, SMAX * EXPW], F16, tag="exw")
                        nc.scalar.activation(
                            out=v(
                                exw[:],
                                [(EXPW * D, ntc), (EXPW, D), (1, EXPW)],
                                off=lc * EXPW,
                            ),
                            in_=v(e1[:], [(D, ntc), (1, D), (0, EXPW)], off=lc),
                            func=AF.Exp,
                            bias=shf_sb[:],
                        )
                return gbuf, exw, eexp

            def stage2a(ci, gbuf, exw, eexp):
                col0, W, parts = sched.calls[ci]
                # denominators for the whole call, then one reciprocal
                den = pw.tile([128, NTC_CALL], F32, tag="den")
                ti = 0
                tis = []
                for (t0, D, ntc, lc, m_pool, t_pool) in parts:
                    tis.append(ti)
                    if m_pool:
                        nc.vector.tensor_reduce(
                            out=den[:, ti : ti + ntc],
                            in_=v(eexp[:], [(D, ntc), (1, D)], off=lc),
                            axis=X,
                            op=OP.add,
                        )
                    else:
                        nc.vector.tensor_reduce(
                            out=den[:, ti : ti + ntc],
                            in_=v(
                                exw[:], [(EXPW * D, ntc), (EXPW, D)], off=lc * EXPW
                            ),
                            axis=X,
                            op=OP.add,
                        )
                    ti += ntc
                rden = pw.tile([128, NTC_CALL], F32, tag="rden")
                nc.vector.reciprocal(out=rden[:, :ti], in_=den[:, :ti])

                def mult(t0, D, ntc, lc, m_pool, t_pool):
                    dims = [(f * D, ntc), (f, D), (1, f)]
                    if m_pool:
                        # fused broadcast multiply on Pool (gpsimd), in place
                        nc.gpsimd.tensor_tensor(
                            out=v(gbuf[:], dims, off=lc * f),
                            in0=v(gbuf[:], dims, off=lc * f),
                            in1=v(eexp[:], [(D, ntc), (1, D), (0, f)], off=lc),
                            op=OP.mult,
                        )
                    else:
                        # f/EXPW passes at DVE 2x, sharing the EXPW expansion
                        for q in range(0, f, EXPW):
                            qd = [(f * D, ntc), (f, D), (1, EXPW)]
                            nc.vector.tensor_tensor(
                                out=v(gbuf[:], qd, off=lc * f + q),
                                in0=v(gbuf[:], qd, off=lc * f + q),
                                in1=v(
                                    exw[:],
                                    [(EXPW * D, ntc), (EXPW, D), (1, EXPW)],
                                    off=lc * EXPW,
                                ),
                                op=OP.mult,
                            )

                def tree(t0, D, ntc, lc, m_pool, t_pool):
                    # halving-tree sum over slots -> slot 0 of each tile
                    eng = nc.gpsimd if t_pool else nc.vector
                    cur = D
                    while cur > 1:
                        if cur % 2:
                            eng.tensor_tensor(
                                out=v(gbuf[:], [(f * D, ntc), (1, f)], off=lc * f),
                                in0=v(gbuf[:], [(f * D, ntc), (1, f)], off=lc * f),
                                in1=v(
                                    gbuf[:],
                                    [(f * D, ntc), (1, f)],
                                    off=(lc + cur - 1) * f,
                                ),
                                op=OP.add,
                            )
                            cur -= 1
                        h = cur // 2
                        eng.tensor_tensor(
                            out=v(gbuf[:], [(f * D, ntc), (f, h), (1, f)], off=lc * f),
                            in0=v(gbuf[:], [(f * D, ntc), (f, h), (1, f)], off=lc * f),
                            in1=v(
                                gbuf[:],
                                [(f * D, ntc), (f, h), (1, f)],
                                off=(lc + h) * f,
                            ),
                            op=OP.add,
                        )
                        cur = h

                # emission order keeps each in-order engine stream unblocked:
                # every engine sees its own ready work (mults) before any
                # instruction that waits on the other engine (mixed trees,
                # normalizes)
                for part in parts:
                    if part[4]:
                        mult(*part)  # Pool mults (need only eexp)
                for part in parts:
                    if not part[4]:
                        mult(*part)  # DVE mults
                for part in parts:
                    if part[4] == part[5]:
                        tree(*part)  # same-engine chains
                for part in parts:
                    if part[4] != part[5]:
                        tree(*part)  # cross-engine trees last
                return rden, tis

            def stage2b(ci, gbuf, exw, eexp, rden, tis):
                nonlocal ps_ch
                col0, W, parts = sched.calls[ci]
                ht = ph.tile([128, NTC_CALL * f], F16, tag="ht")
                for pi, (t0, D, ntc, lc, m_pool, t_pool) in enumerate(parts):
                    ti0 = tis[pi]
                    # normalize (dst-major, fp16, compacted into ht)
                    nc.vector.tensor_tensor(
                        out=v(ht[:], [(f, ntc), (1, f)], off=ti0 * f),
                        in0=v(gbuf[:], [(f * D, ntc), (1, f)], off=lc * f),
                        in1=rden[:, ti0 : ti0 + ntc].to_broadcast([128, ntc, f]),
                        op=OP.mult,
                    )
                    for tl in range(ntc):
                        t = t0 + tl
                        j = t % CHT
                        if j == 0:
                            ps_ch = pps.tile(
                                [f, CHT * 128], F16, tag="psch", space="PSUM"
                            )
                        nc.tensor.transpose(
                            out=ps_ch[:, j * 128 : (j + 1) * 128],
                            in_=ht[:, (ti0 + tl) * f : (ti0 + tl + 1) * f],
                            identity=ident[:],
                        )
                        if j == CHT - 1 or t == NT - 1:
                            flush_chunk(t // CHT, j + 1)

            # 3-stage software pipeline: s1(ci) loads+exp, s2a(ci-1)
            # mult+tree, s2b(ci-2) normalize+transpose+flush — so the
            # in-order DVE stream never waits on a Pool tree of the same
            # call before starting the next call's multiplies
            st1 = {}
            st2 = {}
            ncalls = len(sched.calls)
            for ci in range(ncalls + 2):
                if ci < ncalls:
                    st1[ci] = stage1(ci)
                    if ci == 0:
                        emit_consts()
                if 1 <= ci < ncalls + 1:
                    st2[ci - 1] = stage2a(ci - 1, *st1[ci - 1])
                if ci >= 2:
                    stage2b(ci - 2, *st1[ci - 2], *st2[ci - 2])
    nc.compile()
    return nc


# ---------------------------------------------------------------- driver
_cache = {}
TRACE = False
LAST_HW_NS = []
LAST_RESULTS = []


def _run(nc, in_maps, cores):
    res = bass_utils.run_bass_kernel_spmd(nc, in_maps, core_ids=cores, trace=TRACE)
    if TRACE:
        LAST_RESULTS.append(res)
        if res.exec_time_ns:
            LAST_HW_NS.append(res.exec_time_ns)
    return res


def _get_schedule(edge_index):
    fp = hashlib.sha1(np.ascontiguousarray(edge_index)).hexdigest()
    key = ("sched", fp)
    if key not in _cache:
        _cache[key] = build_schedule(edge_index)
    return _cache[key]


def kernel(x, edge_index, W1, bW1, A1, bA1, W2, bW2, A2, bA2, Wfc, bfc):
    x = np.asarray(x, dtype=np.float32)
    edge_index = np.asarray(edge_index)
    W1 = np.asarray(W1, np.float32)
    bW1 = np.asarray(bW1, np.float32)
    A1 = np.asarray(A1, np.float32)
    bA1 = np.asarray(bA1, np.float32)
    W2 = np.asarray(W2, np.float32)
    bW2 = np.asarray(bW2, np.float32)
    A2 = np.asarray(A2, np.float32)
    bA2 = np.asarray(bA2, np.float32)
    Wfc = np.asarray(Wfc, np.float32)
    bfc = np.asarray(bfc, np.float32)

    sched = _get_schedule(edge_index)
    cores = list(range(N_CORES))

    if "A" not in _cache:
        _cache["A"] = build_progA()
    ncA = _cache["A"]
    inA = []
    x16 = x.astype(np.float16)
    A1cat = np.concatenate([A1[:F], A1[F:]], axis=1)  # [64, 2]
    WA = (W1 @ A1cat).astype(np.float16)  # [128, 2]
    bA2v = (bW1 @ A1cat).reshape(2, 1).astype(np.float32)
    for c in cores:
        xT = np.ascontiguousarray(x16[c * DPC : (c + 1) * DPC].T)
        inA.append(
            {
                "xT": xT,
                "W": W1.astype(np.float16),
                "bW": bW1.reshape(F, 1),
                "WA": WA,
                "bA2": bA2v,
            }
        )
    LAST_HW_NS.clear()
    LAST_RESULTS.clear()
    resA = _run(ncA, inA, cores)
    wh16 = np.concatenate(
        [resA.results[c]["whT"].T for c in cores], axis=0
    )  # [N, 64] f16
    s_all = np.concatenate([resA.results[c]["s"] for c in cores], axis=1)
    si_full, sj_full = s_all[0], s_all[1]

    key = ("B", sched.n_tiles, sched.w_total, tuple(sched.tiles))
    if key not in _cache:
        _cache[key] = build_progB(sched)
    ncB = _cache[key]
    NT = sched.n_tiles

    def launch_B(wh16_full, si_f, sj_f, bA, Wn, bWn, An):
        bA0 = np.float32(bA.reshape(-1)[0])
        WnAs = Wn @ An  # [64, 2]
        wpack = np.zeros((F + 2, F + 3), np.float32)
        wpack[:F, :F] = Wn
        wpack[:F, F : F + 2] = WnAs
        wpack[:F, F + 2] = bWn.reshape(F)
        wpack[F : F + 2, F + 2] = bWn @ An
        inB = []
        shift = np.float32(max(0.0, float(si_f.max() + sj_f.max() + bA0) - 8.0))
        for c in cores:
            e_p, e_col, e_src, e_dstg = sched.edges[c]
            ss = sched.slot_srcs[c]
            t = wh16_full[np.clip(ss, 0, N_NODES - 1)]
            t[ss < 0] = 0
            sj_arr = np.full((128, sched.w_total), np.float32(PAD_SJ))
            sj_arr[e_p, e_col] = sj_f[e_src]
            si_arr = np.zeros((128, sched.w_total), np.float32)
            si_arr[e_p, e_col] = si_f[e_dstg] + bA0
            sjsi = np.empty((128, 2 * sched.w_total), np.float32)
            for (col0, W, _) in sched.calls:
                sjsi[:, 2 * col0 : 2 * col0 + W] = sj_arr[:, col0 : col0 + W]
                sjsi[:, 2 * col0 + W : 2 * (col0 + W)] = si_arr[:, col0 : col0 + W]
            inB.append(
                {
                    "tbl": np.ascontiguousarray(
                        t.reshape(128, sched.w_total * F)
                    ),
                    "sjsi": sjsi,
                    "wpack": wpack,
                    "shiftv": np.full((128, 1), -shift, np.float32),
                }
            )
        res = _run(ncB, inB, cores)
        whn = np.zeros((N_NODES, F), np.float16)
        sn_i = np.zeros(N_NODES, np.float32)
        sn_j = np.zeros(N_NODES, np.float32)
        for c in cores:
            perm = sched.perms[c]
            real = perm >= 0
            gids = c * DPC + perm[real]
            cb = res.results[c]["comb"]
            whn[gids] = cb[:F].T[real]
            sn_i[gids] = cb[F].astype(np.float32)[real]
            sn_j[gids] = cb[F + 1].astype(np.float32)[real]
        return whn, sn_i, sn_j

    As2 = np.ascontiguousarray(np.concatenate([A2[:F], A2[F:]], axis=1))
    wh2, si2, sj2 = launch_B(wh16, si_full, sj_full, bA1, W2, bW2, As2)
    out, _, _ = launch_B(wh2, si2, sj2, bA2, Wfc, bfc, np.zeros((F, 2), np.float32))
    return out.astype(np.float32)


# revision 31
# speedup vs baseline: 1.3956x; 1.0802x over previous
"""GAT (2-layer) on 8 NeuronCores — Bass/Tile kernel.

Strategy (dst-sharded graph parallel, streamed tables):
  - Each core owns 12500 destination nodes, degree-sorted descending into
    98 tiles of 128 dsts. Per-tile slot capacity D = the max degree of the
    tile across all cores (exact, no grid rounding — degrees span 2..36 so
    pad waste is ~1-3%).
  - The halo "gather" is a plain 2D stream: the host packs each core's
    table in exact slot-consumption order ([128 partitions, w_total x 64
    fp16] = one 128B row per slot, zeros at pads), so the kernel issues
    one contiguous dma_start per call instead of dma_gather — no index
    tables, no SWDGE descriptor generation, Pool is freed for compute.
  - Launch A: per-core Wh1^T = (x W1 + b)^T (fp16 out) + attention
    scalars s_i/s_j.
  - Host between launches: packs the slot tables from device-computed Wh
    (fp16 row gather = pure layout), expands s_j / (s_i + bA - shift) to
    slot level (softmax is shift-invariant; keeps exp inside fp16 range),
    and prepares the fused epilogue weights [Wn | Wn@As] (the same
    weight-folding progA uses). Index/layout work only; all value FLOPs
    run on device.
  - Launch B (x2, one per GAT layer): per call (<=128 slot cols):
    stream table + [sj|si] slices, epre = sj+si (DVE), e1 = leaky (Act),
    then per part a static planner balances three engines: exp is either
    broadcast-expanded to feature width on Act with the 2x-rate fp16
    multiply on DVE, or kept per-slot with a fused broadcast multiply on
    Pool (gpsimd). The slot halving-tree sum is likewise split between
    DVE and Pool. Segment softmax denominators reduce on DVE; normalize
    on DVE; leaky of the layer output rides the PSUM->SBUF Act copy; the
    epilogue is one fused matmul [Wn | Wn@As] producing whn and the next
    layer's attention scalars in a single fp16 output tensor.
"""

import dataclasses
import hashlib
import numpy as np

import concourse.bacc as bacc
import concourse.tile as tile
from concourse import bass, mybir, bass_utils
from concourse.masks import make_identity

F32 = mybir.dt.float32
F16 = mybir.dt.float16

N_NODES = 100000
N_CORES = 8
DPC = N_NODES // N_CORES
F = 64
IN_C = 128
SMAX = 128  # slot columns per call
EXPW = 32  # exp expansion width; DVE multiplies in F/EXPW passes
CHT = 4  # tiles per epilogue chunk (512 dsts)
PAD_SJ = -60.0
ALPHA = 0.2

# planner cost constants (ns); see TRN2Spec in concourse/hw_specs.py
ACT_EL = 0.833
DVE2X = 0.5208
DVE1X = 1.0417
POOL_EL = 1.984  # 0.833 / 0.42 gpsimd Add/Multiply efficiency
FIX_ACT = 185.0
FIX_DVE = 146.0
FIX_POOL = 60.0


@dataclasses.dataclass
class Schedule:
    n_tiles: int
    w_total: int  # slot columns
    tiles: list  # per tile: D
    calls: list  # (col0, W, parts) ; parts: [(t0, D, ntc, lc, m_pool, t_pool)]
    perms: list  # per core: int64 [n_tiles*128], local dst or -1
    slot_srcs: list  # per core: int32 [128, w_total], global src or -1
    edges: list  # per core: (e_p, e_col, e_src, e_dstg) for sj/si expansion


def build_schedule(edge_index: np.ndarray) -> Schedule:
    src = np.asarray(edge_index[0], dtype=np.int64)
    dst = np.asarray(edge_index[1], dtype=np.int64)
    E = src.shape[0]
    order = np.argsort(dst, kind="stable")
    src_s = src[order]
    dst_s = dst[order]
    deg_all = np.bincount(dst, minlength=N_NODES)
    starts_all = np.concatenate([[0], np.cumsum(deg_all)])
    k_s = np.arange(E) - starts_all[dst_s]

    nt = -(-DPC // 128)
    # degree-descending per core; shared tile capacity = max over cores
    perms = []
    tmax = np.zeros(nt)
    for c in range(N_CORES):
        deg = deg_all[c * DPC : (c + 1) * DPC]
        rank = np.argsort(deg, kind="stable")[::-1]
        perm = np.full(nt * 128, -1, dtype=np.int64)
        perm[:DPC] = rank
        perms.append(perm)
        d = np.where(perm >= 0, deg[np.clip(perm, 0, DPC - 1)], 0)
        tmax = np.maximum(tmax, d.reshape(nt, 128).max(1))
    tiles = [int(x) for x in tmax]
    n_tiles = nt

    # pack runs of equal D into calls of <= SMAX slot columns (ramped)
    runs = []
    i = 0
    while i < n_tiles:
        D = tiles[i]
        n = 1
        while i + n < n_tiles and tiles[i + n] == D:
            n += 1
        runs.append((i, D, n))
        i += n
    calls = []  # (col0, W, parts)
    cur, cw = [], 0
    for (t0, D, n) in runs:
        rem_t0, rem_n = t0, n
        while rem_n:
            lim = (32, 64)[len(calls)] if len(calls) < 2 else SMAX
            lim = max(lim, D)
            if cur and cw + D > lim:
                calls.append((0, cw, cur))
                cur, cw = [], 0
                continue
            take = min(rem_n, (lim - cw) // D)
            cur.append((rem_t0, D, take, cw))
            cw += D * take
            rem_t0 += take
            rem_n -= take
    if cur:
        calls.append((0, cw, cur))
    # split a small tail off the last call so the drain is short
    _, wl, parts_l = calls[-1]
    if wl > 32:
        target = wl - 16
        p1, p2, acc, w1, w2 = [], [], 0, 0, 0
        for (t0, D, ntc, lc) in parts_l:
            for tl in range(ntc):
                dstp, w = (p1, w1) if acc < target else (p2, w2)
                if dstp and dstp[-1][0] + dstp[-1][2] == t0 + tl and dstp[-1][1] == D:
                    dstp[-1] = (dstp[-1][0], D, dstp[-1][2] + 1, dstp[-1][3])
                else:
                    dstp.append((t0 + tl, D, 1, w))
                if acc < target:
                    w1 += D
                else:
                    w2 += D
                acc += D
        calls[-1] = (0, w1, [tuple(x) for x in p1])
        calls.append((0, w2, [tuple(x) for x in p2]))
    col = 0
    for j, (_, W, parts) in enumerate(calls):
        calls[j] = (col, W, parts)
        col += W
    w_total = col

    # chop parts into <=3-tile units so the engine planner can split work
    # within single-part calls
    chopped = []
    for (col0, W, parts) in calls:
        np_ = []
        for (t0, D, ntc, lc) in parts:
            o = 0
            while o < ntc:
                k = min(3, ntc - o)
                np_.append((t0 + o, D, k, lc + o * D))
                o += k
        chopped.append((col0, W, np_))
    calls = chopped

    # --- static engine planner: balance Act / DVE / Pool per call ---------
    # (local balance — global-only balancing lets consecutive parts pile on
    # one engine and the in-order engine streams seesaw)
    planned_calls = []
    flush_per_tile = 2 * (FIX_ACT + 512 * ACT_EL) / CHT
    ncalls_total = len(calls)
    for cidx, (col0, W, parts) in enumerate(calls):
        # discourage Pool near the drain: its lag has nothing to overlap
        pool_el = POOL_EL * (1.7 if cidx >= ncalls_total - 3 else 1.0)
        ntc_call = sum(p[2] for p in parts)
        actT = FIX_ACT + W * ACT_EL + ntc_call * flush_per_tile  # e1 + flush
        dveT = 2 * FIX_DVE + W * DVE1X  # epre + reciprocal
        poolT = 0.0
        newparts = []
        for (t0, D, ntc, lc) in parts:
            S = D * ntc
            Eel = 64.0 * S
            Tel = 64.0 * (S - ntc)
            lev = max(1, int(np.ceil(np.log2(max(D, 2)))))
            dveT += FIX_DVE + S * DVE1X  # den reduce
            dveT += FIX_DVE + ntc * 64 * DVE1X  # normalize
            best = None
            for m_pool in (0, 1):
                for t_pool in (0, 1):
                    a, d, p = actT, dveT, poolT
                    if m_pool:
                        a += FIX_ACT + S * ACT_EL
                        p += FIX_POOL + Eel * pool_el
                    else:
                        a += FIX_ACT + EXPW * S * ACT_EL
                        d += (64 // EXPW) * FIX_DVE + Eel * DVE2X
                    if t_pool:
                        p += lev * FIX_POOL + Tel * pool_el
                    else:
                        d += lev * FIX_DVE + Tel * DVE2X
                    mk = max(a, d, p)
                    if best is None or mk < best[0]:
                        best = (mk, m_pool, t_pool, a, d, p)
            _, m_pool, t_pool, actT, dveT, poolT = best
            newparts.append((t0, D, ntc, lc, m_pool, t_pool))
        planned_calls.append((col0, W, newparts))
    calls = planned_calls

    # --- per-core slot-level source map ----------------------------------
    tile_col0 = np.zeros(n_tiles, np.int64)
    for (col0, W, parts) in calls:
        for (t0, D, ntc, lc, _, _) in parts:
            for tl in range(ntc):
                tile_col0[t0 + tl] = col0 + lc + tl * D

    slot_srcs, edges = [], []
    for c in range(N_CORES):
        perm = perms[c]
        real = perm >= 0
        pos_of_dst = np.empty(DPC, np.int64)
        pos_of_dst[perm[real]] = np.flatnonzero(real)
        gtile_of_dst = pos_of_dst // 128
        p_of_dst = pos_of_dst % 128

        lo, hi = starts_all[c * DPC], starts_all[(c + 1) * DPC]
        e_src = src_s[lo:hi].astype(np.int32)
        e_dstl = dst_s[lo:hi] - c * DPC
        e_k = k_s[lo:hi]
        e_tile = gtile_of_dst[e_dstl]
        e_p = p_of_dst[e_dstl].astype(np.int32)
        e_col = (tile_col0[e_tile] + e_k).astype(np.int32)
        S = np.full((128, w_total), -1, np.int32)
        S[e_p, e_col] = e_src
        slot_srcs.append(S)
        edges.append((e_p, e_col, e_src, (c * DPC + e_dstl).astype(np.int32)))

    return Schedule(n_tiles, w_total, tiles, calls, perms, slot_srcs, edges)


# ---------------------------------------------------------------- prog A
def build_progA(n_loc=DPC, in_c=IN_C, f=F):
    # one fused matmul per 512-col chunk: lhsT = [W1 | W1@A1cat] so Wh and
    # both attention scalars come out of a single PSUM tile / Act copy
    nc = bacc.Bacc("TRN2", target_bir_lowering=False, debug=False, num_devices=N_CORES)
    xT = nc.dram_tensor("xT", [in_c, n_loc], F16, kind="ExternalInput").ap()
    WP = nc.dram_tensor("WP", [in_c, f + 2], F16, kind="ExternalInput").ap()
    bP = nc.dram_tensor("bP", [f + 2, 1], F32, kind="ExternalInput").ap()
    combA = nc.dram_tensor("combA", [f + 2, n_loc], F16, kind="ExternalOutput").ap()

    with tile.TileContext(nc) as tc:
        with tc.tile_pool(name="sb", bufs=1) as pool, tc.tile_pool(
            name="ps", bufs=4, space="PSUM"
        ) as pps, tc.tile_pool(name="sb2", bufs=3) as pool2:
            WP_sb = pool.tile([in_c, f + 2], F16)
            nc.sync.dma_start(out=WP_sb[:], in_=WP[:, :])
            bP_sb = pool.tile([f + 2, 1], F32)
            nc.sync.dma_start(out=bP_sb[:], in_=bP[:, :])
            xT_sb = pool.tile([in_c, n_loc], F16)
            XCH = 1563
            for x0 in range(0, n_loc, XCH):
                xc = min(XCH, n_loc - x0)
                nc.sync.dma_start(
                    out=xT_sb[:, x0 : x0 + xc], in_=xT[:, x0 : x0 + xc]
                )

            CH = 512
            GRP = 4  # store in 2048-column groups
            wh_g = None
            for c0 in range(0, n_loc, CH):
                ch = min(CH, n_loc - c0)
                gi = (c0 // CH) % GRP
                if gi == 0:
                    wh_g = pool2.tile([f + 2, GRP * CH], F16, tag="whg")
                ps_w = pps.tile([f + 2, CH], F32, space="PSUM")
                nc.tensor.matmul(
                    out=ps_w[:, :ch],
                    lhsT=WP_sb[:],
                    rhs=xT_sb[:, c0 : c0 + ch],
                    start=True,
                    stop=True,
                )
                if (c0 // CH) % 2 == 0:
                    # alternate the PSUM->SBUF copy between Act and DVE
                    nc.scalar.activation(
                        out=wh_g[:, gi * CH : gi * CH + ch],
                        in_=ps_w[:, :ch],
                        func=mybir.ActivationFunctionType.Identity,
                        bias=bP_sb[:],
                    )
                else:
                    nc.vector.tensor_scalar(
                        out=wh_g[:, gi * CH : gi * CH + ch],
                        in0=ps_w[:, :ch],
                        scalar1=bP_sb[:, 0:1],
                        scalar2=None,
                        op0=mybir.AluOpType.add,
                    )
                if gi == GRP - 1 or c0 + ch >= n_loc:
                    g0 = (c0 // CH // GRP) * GRP * CH
                    gl = c0 + ch - g0
                    nc.sync.dma_start(
                        out=combA[:, g0 : g0 + gl], in_=wh_g[:, :gl]
                    )
    nc.compile()
    return nc


# ---------------------------------------------------------------- prog B
def build_progB(sched: Schedule, f=F):
    NT = sched.n_tiles
    WTOT = sched.w_total
    nc = bacc.Bacc("TRN2", target_bir_lowering=False, debug=False, num_devices=N_CORES)
    tbl = nc.dram_tensor("tbl", [128, WTOT * f], F16, kind="ExternalInput").ap()
    sjsi = nc.dram_tensor("sjsi", [128, 2 * WTOT], F32, kind="ExternalInput").ap()
    # packed: rows 0-63 x cols 0-65 = [Wn | Wn@As]; col 66 = bias (66 rows)
    wp_d = nc.dram_tensor("wpack", [f + 2, f + 3], F32, kind="ExternalInput").ap()
    # [128, 1] replicated -shift, applied as the exp bias (post-leaky, so
    # softmax shift-invariance holds exactly)
    shf_d = nc.dram_tensor("shiftv", [128, 1], F32, kind="ExternalInput").ap()
    comb = nc.dram_tensor("comb", [f + 2, NT * 128], F16, kind="ExternalOutput").ap()

    X = mybir.AxisListType.X
    AF = mybir.ActivationFunctionType
    OP = mybir.AluOpType

    def v(ap, dims, off=0):
        return dataclasses.replace(
            ap,
            ap=[list(ap.ap[0])] + [list(d) for d in dims],
            offset=ap.offset + off,
        )

    NTCMAX = max(
        ntc for (_, _, parts) in sched.calls for (_, _, ntc, _, _, _) in parts
    )
    NTC_CALL = max(
        sum(ntc for (_, _, ntc, _, _, _) in parts) for (_, _, parts) in sched.calls
    )

    with tile.TileContext(nc) as tc:
        with tc.tile_pool(name="const", bufs=1) as pc, tc.tile_pool(
            name="gat", bufs=5
        ) as pg, tc.tile_pool(name="exw", bufs=4) as px, tc.tile_pool(
            name="work", bufs=5
        ) as pw, tc.tile_pool(name="ht", bufs=3) as ph, tc.tile_pool(
            name="ps", bufs=2, space="PSUM"
        ) as pps, tc.tile_pool(name="ep", bufs=3) as pep:
            wp_sb = pc.tile([f + 2, f + 3], F32)
            WC_sb = pc.tile([f, f + 2], F16)
            bias_sb = wp_sb[:, f + 2 : f + 3]
            ident = pc.tile([128, 128], F16)
            shf_sb = pc.tile([128, 1], F32)
            nc.sync.dma_start(out=shf_sb[:], in_=shf_d[:, :])

            def emit_consts():
                # deferred past the first call's stream so the startup HWDGE
                # FIFO isn't serialized ahead of it
                nc.sync.dma_start(out=wp_sb[:], in_=wp_d[:, :])
                nc.vector.tensor_scalar(
                    out=WC_sb[:],
                    in0=wp_sb[: f, : f + 2],
                    scalar1=1.0,
                    scalar2=None,
                    op0=OP.mult,
                )
                make_identity(nc, ident[:])

            ps_ch = None

            def flush_chunk(ck, ntl):
                cols = ntl * 128
                hTL = pep.tile([f, CHT * 128], F16, tag="hTL")
                nc.scalar.activation(
                    out=hTL[:, :cols],
                    in_=ps_ch[:, :cols],
                    func=AF.Prelu,
                    alpha=ALPHA,
                )
                ps_c = pps.tile([f + 2, CHT * 128], F32, tag="psc", space="PSUM")
                nc.tensor.matmul(
                    out=ps_c[:, :cols],
                    lhsT=WC_sb[:],
                    rhs=hTL[:, :cols],
                    start=True,
                    stop=True,
                )
                c_sb = pep.tile([f + 2, CHT * 128], F16, tag="csb")
                nc.scalar.activation(
                    out=c_sb[:, :cols],
                    in_=ps_c[:, :cols],
                    func=AF.Identity,
                    bias=bias_sb,
                )
                nc.sync.dma_start(
                    out=comb[:, ck * CHT * 128 : ck * CHT * 128 + cols],
                    in_=c_sb[:, :cols],
                )

            def stage1(ci):
                col0, W, parts = sched.calls[ci]
                sj_sb = pw.tile([128, 2 * SMAX], F32, tag="sjsi")
                nc.sync.dma_start(
                    out=sj_sb[:, : 2 * W], in_=sjsi[:, 2 * col0 : 2 * col0 + 2 * W]
                )
                gbuf = pg.tile([128, SMAX * f], F16, tag="gbuf")
                nc.sync.dma_start(
                    out=gbuf[:, : W * f], in_=tbl[:, col0 * f : (col0 + W) * f]
                )
                # epre = sj + si' (si' = si + bA - shift; sj = -60 at pads)
                epre = pw.tile([128, SMAX], F32, tag="epre")
                nc.vector.tensor_tensor(
                    out=epre[:, :W],
                    in0=sj_sb[:, :W],
                    in1=sj_sb[:, W : 2 * W],
                    op=OP.add,
                )
                # e = leaky_relu(epre) on Act
                e1 = pw.tile([128, SMAX], F32, tag="e1")
                nc.scalar.activation(
                    out=e1[:, :W], in_=epre[:, :W], func=AF.Prelu, alpha=ALPHA
                )
                exw = None
                eexp = None
                for (t0, D, ntc, lc, m_pool, t_pool) in parts:
                    if m_pool:
                        # per-slot exp only; Pool does the fused multiply
                        if eexp is None:
                            eexp = pw.tile([128, SMAX], F16, tag="eexp")
                        nc.scalar.activation(
                            out=eexp[:, lc : lc + D * ntc],
                            in_=e1[:, lc : lc + D * ntc],
                            func=AF.Exp,
                            bias=shf_sb[:],
                        )
                    else:
                        # exp broadcast-expanded to EXPW on Act; the DVE
                        # multiply re-reads it f/EXPW times at 2x rate
                        if exw is None:
                            exw = px.tile([128, SMAX * EXPW], F16, tag="exw")
                        nc.scalar.activation(
                            out=v(
                                exw[:],
                                [(EXPW * D, ntc), (EXPW, D), (1, EXPW)],
                                off=lc * EXPW,
                            ),
                            in_=v(e1[:], [(D, ntc), (1, D), (0, EXPW)], off=lc),
                            func=AF.Exp,
                            bias=shf_sb[:],
                        )
                return gbuf, exw, eexp

            def stage2a(ci, gbuf, exw, eexp):
                col0, W, parts = sched.calls[ci]
                # denominators for the whole call, then one reciprocal
                den = pw.tile([128, NTC_CALL], F32, tag="den")
                ti = 0
                tis = []
                for (t0, D, ntc, lc, m_pool, t_pool) in parts:
                    tis.append(ti)
                    if m_pool:
                        nc.vector.tensor_reduce(
                            out=den[:, ti : ti + ntc],
                            in_=v(eexp[:], [(D, ntc), (1, D)], off=lc),
                            axis=X,
                            op=OP.add,
                        )
                    else:
                        nc.vector.tensor_reduce(
                            out=den[:, ti : ti + ntc],
                            in_=v(
                                exw[:], [(EXPW * D, ntc), (EXPW, D)], off=lc * EXPW
                            ),
                            axis=X,
                            op=OP.add,
                        )
                    ti += ntc
                rden = pw.tile([128, NTC_CALL], F32, tag="rden")
                nc.vector.reciprocal(out=rden[:, :ti], in_=den[:, :ti])

                def mult(t0, D, ntc, lc, m_pool, t_pool):
                    dims = [(f * D, ntc), (f, D), (1, f)]
                    if m_pool:
                        # fused broadcast multiply on Pool (gpsimd), in place
                        nc.gpsimd.tensor_tensor(
                            out=v(gbuf[:], dims, off=lc * f),
                            in0=v(gbuf[:], dims, off=lc * f),
                            in1=v(eexp[:], [(D, ntc), (1, D), (0, f)], off=lc),
                            op=OP.mult,
                        )
                    else:
                        # f/EXPW passes at DVE 2x, sharing the EXPW expansion
                        for q in range(0, f, EXPW):
                            qd = [(f * D, ntc), (f, D), (1, EXPW)]
                            nc.vector.tensor_tensor(
                                out=v(gbuf[:], qd, off=lc * f + q),
                                in0=v(gbuf[:], qd, off=lc * f + q),
                                in1=v(
                                    exw[:],
                                    [(EXPW * D, ntc), (EXPW, D), (1, EXPW)],
                                    off=lc * EXPW,
                                ),
                                op=OP.mult,
                            )

                def tree(t0, D, ntc, lc, m_pool, t_pool):
                    # halving-tree sum over slots -> slot 0 of each tile
                    eng = nc.gpsimd if t_pool else nc.vector
                    cur = D
                    while cur > 1:
                        if cur % 2:
                            eng.tensor_tensor(
                                out=v(gbuf[:], [(f * D, ntc), (1, f)], off=lc * f),
                                in0=v(gbuf[:], [(f * D, ntc), (1, f)], off=lc * f),
                                in1=v(
                                    gbuf[:],
                                    [(f * D, ntc), (1, f)],
                                    off=(lc + cur - 1) * f,
                                ),
                                op=OP.add,
                            )
                            cur -= 1
                        h = cur // 2
                        eng.tensor_tensor(
                            out=v(gbuf[:], [(f * D, ntc), (f, h), (1, f)], off=lc * f),
                            in0=v(gbuf[:], [(f * D, ntc), (f, h), (1, f)], off=lc * f),
                            in1=v(
                                gbuf[:],
                                [(f * D, ntc), (f, h), (1, f)],
                                off=(lc + h) * f,
                            ),
                            op=OP.add,
                        )
                        cur = h

                # emission order keeps each in-order engine stream unblocked:
                # every engine sees its own ready work (mults) before any
                # instruction that waits on the other engine (mixed trees,
                # normalizes)
                for part in parts:
                    if part[4]:
                        mult(*part)  # Pool mults (need only eexp)
                for part in parts:
                    if not part[4]:
                        mult(*part)  # DVE mults
                for part in parts:
                    if part[4] == part[5]:
                        tree(*part)  # same-engine chains
                for part in parts:
                    if part[4] != part[5]:
                        tree(*part)  # cross-engine trees last
                return rden, tis

            def stage2b(ci, gbuf, exw, eexp, rden, tis):
                nonlocal ps_ch
                col0, W, parts = sched.calls[ci]
                ht = ph.tile([128, NTC_CALL * f], F16, tag="ht")
                for pi, (t0, D, ntc, lc, m_pool, t_pool) in enumerate(parts):
                    ti0 = tis[pi]
                    # normalize (dst-major, fp16, compacted into ht)
                    nc.vector.tensor_tensor(
                        out=v(ht[:], [(f, ntc), (1, f)], off=ti0 * f),
                        in0=v(gbuf[:], [(f * D, ntc), (1, f)], off=lc * f),
                        in1=rden[:, ti0 : ti0 + ntc].to_broadcast([128, ntc, f]),
                        op=OP.mult,
                    )
                    for tl in range(ntc):
                        t = t0 + tl
                        j = t % CHT
                        if j == 0:
                            ps_ch = pps.tile(
                                [f, CHT * 128], F16, tag="psch", space="PSUM"
                            )
                        nc.tensor.transpose(
                            out=ps_ch[:, j * 128 : (j + 1) * 128],
                            in_=ht[:, (ti0 + tl) * f : (ti0 + tl + 1) * f],
                            identity=ident[:],
                        )
                        if j == CHT - 1 or t == NT - 1:
                            flush_chunk(t // CHT, j + 1)

            # 4-deep software pipeline: s1(ci) loads+exp two calls ahead,
            # s2a(ci-2) mult+tree, s2b(ci-3) normalize+transpose+flush —
            # the in-order DVE stream never waits on a Pool tree of the
            # same call before starting the next call's multiplies, and
            # loads/exps run well ahead of consumption
            st1 = {}
            st2 = {}
            ncalls = len(sched.calls)
            for ci in range(ncalls + 3):
                if ci < ncalls:
                    st1[ci] = stage1(ci)
                    if ci == 0:
                        emit_consts()
                if 2 <= ci < ncalls + 2:
                    st2[ci - 2] = stage2a(ci - 2, *st1[ci - 2])
                if ci >= 3:
                    stage2b(ci - 3, *st1[ci - 3], *st2[ci - 3])
    nc.compile()
    return nc


# ---------------------------------------------------------------- driver
_cache = {}
TRACE = False
LAST_HW_NS = []
LAST_RESULTS = []


def _run(nc, in_maps, cores):
    res = bass_utils.run_bass_kernel_spmd(nc, in_maps, core_ids=cores, trace=TRACE)
    if TRACE:
        LAST_RESULTS.append(res)
        if res.exec_time_ns:
            LAST_HW_NS.append(res.exec_time_ns)
    return res


def _get_schedule(edge_index):
    fp = hashlib.sha1(np.ascontiguousarray(edge_index)).hexdigest()
    key = ("sched", fp)
    if key not in _cache:
        _cache[key] = build_schedule(edge_index)
    return _cache[key]


def kernel(x, edge_index, W1, bW1, A1, bA1, W2, bW2, A2, bA2, Wfc, bfc):
    x = np.asarray(x, dtype=np.float32)
    edge_index = np.asarray(edge_index)
    W1 = np.asarray(W1, np.float32)
    bW1 = np.asarray(bW1, np.float32)
    A1 = np.asarray(A1, np.float32)
    bA1 = np.asarray(bA1, np.float32)
    W2 = np.asarray(W2, np.float32)
    bW2 = np.asarray(bW2, np.float32)
    A2 = np.asarray(A2, np.float32)
    bA2 = np.asarray(bA2, np.float32)
    Wfc = np.asarray(Wfc, np.float32)
    bfc = np.asarray(bfc, np.float32)

    sched = _get_schedule(edge_index)
    cores = list(range(N_CORES))

    if "A" not in _cache:
        _cache["A"] = build_progA()
    ncA = _cache["A"]
    inA = []
    x16 = x.astype(np.float16)
    A1cat = np.concatenate([A1[:F], A1[F:]], axis=1)  # [64, 2]
    WP = np.concatenate([W1, W1 @ A1cat], axis=1).astype(np.float16)  # [128, 66]
    bP = np.concatenate([bW1, bW1 @ A1cat]).reshape(F + 2, 1).astype(np.float32)
    for c in cores:
        xT = np.ascontiguousarray(x16[c * DPC : (c + 1) * DPC].T)
        inA.append({"xT": xT, "WP": WP, "bP": bP})
    LAST_HW_NS.clear()
    LAST_RESULTS.clear()
    resA = _run(ncA, inA, cores)
    cA = np.concatenate([resA.results[c]["combA"] for c in cores], axis=1)
    wh16 = np.ascontiguousarray(cA[:F].T)  # [N, 64] f16
    si_full = cA[F].astype(np.float32)
    sj_full = cA[F + 1].astype(np.float32)

    key = ("B", sched.n_tiles, sched.w_total, tuple(sched.tiles))
    if key not in _cache:
        _cache[key] = build_progB(sched)
    ncB = _cache[key]
    NT = sched.n_tiles

    def launch_B(wh16_full, si_f, sj_f, bA, Wn, bWn, An):
        bA0 = np.float32(bA.reshape(-1)[0])
        WnAs = Wn @ An  # [64, 2]
        wpack = np.zeros((F + 2, F + 3), np.float32)
        wpack[:F, :F] = Wn
        wpack[:F, F : F + 2] = WnAs
        wpack[:F, F + 2] = bWn.reshape(F)
        wpack[F : F + 2, F + 2] = bWn @ An
        inB = []
        shift = np.float32(max(0.0, float(si_f.max() + sj_f.max() + bA0) - 8.0))
        for c in cores:
            e_p, e_col, e_src, e_dstg = sched.edges[c]
            ss = sched.slot_srcs[c]
            t = wh16_full[np.clip(ss, 0, N_NODES - 1)]
            t[ss < 0] = 0
            sj_arr = np.full((128, sched.w_total), np.float32(PAD_SJ))
            sj_arr[e_p, e_col] = sj_f[e_src]
            si_arr = np.zeros((128, sched.w_total), np.float32)
            si_arr[e_p, e_col] = si_f[e_dstg] + bA0
            sjsi = np.empty((128, 2 * sched.w_total), np.float32)
            for (col0, W, _) in sched.calls:
                sjsi[:, 2 * col0 : 2 * col0 + W] = sj_arr[:, col0 : col0 + W]
                sjsi[:, 2 * col0 + W : 2 * (col0 + W)] = si_arr[:, col0 : col0 + W]
            inB.append(
                {
                    "tbl": np.ascontiguousarray(
                        t.reshape(128, sched.w_total * F)
                    ),
                    "sjsi": sjsi,
                    "wpack": wpack,
                    "shiftv": np.full((128, 1), -shift, np.float32),
                }
            )
        res = _run(ncB, inB, cores)
        whn = np.zeros((N_NODES, F), np.float16)
        sn_i = np.zeros(N_NODES, np.float32)
        sn_j = np.zeros(N_NODES, np.float32)
        for c in cores:
            perm = sched.perms[c]
            real = perm >= 0
            gids = c * DPC + perm[real]
            cb = res.results[c]["comb"]
            whn[gids] = cb[:F].T[real]
            sn_i[gids] = cb[F].astype(np.float32)[real]
            sn_j[gids] = cb[F + 1].astype(np.float32)[real]
        return whn, sn_i, sn_j

    As2 = np.ascontiguousarray(np.concatenate([A2[:F], A2[F:]], axis=1))
    wh2, si2, sj2 = launch_B(wh16, si_full, sj_full, bA1, W2, bW2, As2)
    out, _, _ = launch_B(wh2, si2, sj2, bA2, Wfc, bfc, np.zeros((F, 2), np.float32))
    return out.astype(np.float32)


# revision 43
# speedup vs baseline: 1.4689x; 1.0525x over previous
"""GAT (2-layer) on 8 NeuronCores — Bass/Tile kernel.

Strategy (dst-sharded graph parallel, PE segment-sum):
  - Each core owns 12500 destination nodes, degree-sorted descending into
    98 tiles of 128 dsts. Per-tile slot capacity c = a near-exact cap on
    the tile's max degree chosen so G = 128//c destination nodes pack one
    128-partition block (partition p = dst g=p//c, slot s=p%c).
  - The halo "gather" is a plain 2D stream: the host packs each core's
    cells in exact consumption order ([128, blocks x 65 fp16]; 64 feature
    cols per block plus a hole at col 64 for exp), so the kernel issues
    one contiguous dma_start per call — no dma_gather, no index tables.
  - The segment softmax sum runs on the Tensor engine: per block, one
    matmul with lhsT = the 65-col data block and rhs = a constant
    block-diagonal ones matrix produces out[65, G] — feature-major
    (already transposed for the epilogue) with the softmax denominator
    riding as row 64. Disjoint psum columns per block; no tree, no
    accumulation hazards.
  - Per-edge exp weights: e1 = leaky(sj+si) (Act), exp written into the
    col-64 holes (Act), then the feature multiply is split between
    [Act 32-wide exp broadcast-expand + DVE 2x-rate fp16 multiply] and a
    fused stride-0 broadcast multiply on Pool (gpsimd), balanced by a
    static per-call planner.
  - Normalization: rden = recip(den row), partition-broadcast by a
    1-partition ones matmul into PSUM, multiplied after the leaky
    PSUM->SBUF copy (positive scales commute with leaky-relu). The
    epilogue is one fused matmul [Wn | Wn@As] producing whn and the next
    layer's attention scalars in a single fp16 output tensor.
  - Launch A: one fused matmul per chunk with lhsT = [W1 | W1@A1cat].
  - Host between launches does index/layout work only (cell packing from
    device-computed Wh, sj/si expansion, weight folding); all value
    FLOPs run on device.
"""

import dataclasses
import hashlib
import numpy as np

import concourse.bacc as bacc
import concourse.tile as tile
from concourse import bass, mybir, bass_utils

F32 = mybir.dt.float32
F16 = mybir.dt.float16

N_NODES = 100000
N_CORES = 8
DPC = N_NODES // N_CORES
F = 64
IN_C = 128
BMAX = 160  # blocks per call
EXPW = 32  # exp expansion width; DVE multiplies in F/EXPW passes
CHT = 4  # tiles per epilogue chunk (512 dsts)
PAD_SJ = -60.0
ALPHA = 0.2
SHIFT_TARGET = 6.0  # keep exp<=e^6 so f16 denominators can't overflow

# planner cost constants (ns); see TRN2Spec in concourse/hw_specs.py
ACT_EL = 0.833
DVE2X = 0.5208
DVE1X = 1.0417
POOL_EL = 1.984  # 0.833 / 0.42 gpsimd Add/Multiply efficiency
FIX_ACT = 185.0
FIX_DVE = 146.0
FIX_POOL = 60.0


@dataclasses.dataclass
class Schedule:
    n_tiles: int
    tot_blk: int
    tiles: list  # per tile: (cap, G, nblk_t, bstart)
    blocks: list  # per block: (tile, kb, gg, dcol, gcol0)
    calls: list  # (b0, nblk, parts) ; parts: [(pb0, nb, m_pool)]
    caps: list  # [(cap, G, gcol0)]
    ones_cols: int
    perms: list  # per core: int64 [n_tiles*128], local dst or -1
    cell_src: list  # per core: int32 [128, tot_blk], global src or -1
    cell_dst: list  # per core: int32 [128, tot_blk], global dst or -1


def _best_cap(D):
    best = None
    for c in range(max(2, D), 129):
        G = 128 // c
        if G == 0:
            break
        ov = (128.0 / G) / D
        if best is None or ov < best[0]:
            best = (ov, c, G)
    return best[1], best[2]


def build_schedule(edge_index: np.ndarray) -> Schedule:
    src = np.asarray(edge_index[0], dtype=np.int64)
    dst = np.asarray(edge_index[1], dtype=np.int64)
    E = src.shape[0]
    order = np.argsort(dst, kind="stable")
    src_s = src[order]
    dst_s = dst[order]
    deg_all = np.bincount(dst, minlength=N_NODES)
    starts_all = np.concatenate([[0], np.cumsum(deg_all)])
    k_s = np.arange(E) - starts_all[dst_s]

    nt = -(-DPC // 128)
    perms = []
    tmax = np.zeros(nt)
    for c in range(N_CORES):
        deg = deg_all[c * DPC : (c + 1) * DPC]
        rank = np.argsort(deg, kind="stable")[::-1]
        perm = np.full(nt * 128, -1, dtype=np.int64)
        perm[:DPC] = rank
        perms.append(perm)
        d = np.where(perm >= 0, deg[np.clip(perm, 0, DPC - 1)], 0)
        tmax = np.maximum(tmax, d.reshape(nt, 128).max(1))
    n_tiles = nt

    caps_used = {}
    tiles = []
    blocks = []
    bstart = 0
    for t in range(n_tiles):
        D = int(tmax[t])
        cap, G = _best_cap(D)
        if cap not in caps_used:
            caps_used[cap] = G
        nblk_t = -(-128 // G)
        tiles.append((cap, G, nblk_t, bstart))
        for kb in range(nblk_t):
            gg = min(G, 128 - kb * G)
            dcol = (t % CHT) * 128 + kb * G
            blocks.append((t, kb, gg, dcol, cap))
        bstart += nblk_t
    tot_blk = bstart

    caps = []
    gcol = 0
    capmap = {}
    for cap in sorted(caps_used):
        G = caps_used[cap]
        caps.append((cap, G, gcol))
        capmap[cap] = gcol
        gcol += G
    ones_cols = gcol
    blocks = [(t, kb, gg, dcol, capmap[cap]) for (t, kb, gg, dcol, cap) in blocks]

    # pack whole tiles into calls of <= BMAX blocks (ramped starts)
    calls = []
    b0 = 0
    t = 0
    while t < n_tiles:
        lim = (24, 48)[len(calls)] if len(calls) < 2 else BMAX
        nblk = 0
        while t < n_tiles and nblk + tiles[t][2] <= lim:
            nblk += tiles[t][2]
            t += 1
        if nblk == 0:  # single tile larger than lim
            nblk = tiles[t][2]
            t += 1
        calls.append((b0, nblk, None))
        b0 += nblk

    # --- static engine planner: split the multiply DVE vs Pool per call ---
    planned = []
    flushA = 2 * (FIX_ACT + 512 * ACT_EL) / CHT
    flushD = (FIX_DVE + 512 * DVE1X + FIX_DVE + 512 * DVE2X) / CHT
    ncalls_total = len(calls)
    for cidx, (b0, nblk, _) in enumerate(calls):
        pool_el = POOL_EL * (1.7 if cidx >= ncalls_total - 2 else 1.0)
        ntl = sum(1 for (tt, kb, _, _, _) in blocks[b0 : b0 + nblk] if kb == 0)
        actT = 2 * FIX_ACT + 2 * nblk * ACT_EL + ntl * flushA  # e1+exp+flush
        dveT = FIX_DVE + nblk * DVE1X + ntl * flushD  # epre + flush
        poolT = 0.0
        parts = []
        pb = 0
        while pb < nblk:
            nb = min(12, nblk - pb)
            a_x = actT + FIX_ACT + nb * EXPW * ACT_EL
            d_x = dveT + (64 // EXPW) * FIX_DVE + nb * 64 * DVE2X
            p_y = poolT + FIX_POOL + nb * 64 * pool_el
            if max(a_x, d_x, poolT) <= max(actT, dveT, p_y):
                actT, dveT = a_x, d_x
                parts.append((pb, nb, 0))
            else:
                poolT = p_y
                parts.append((pb, nb, 1))
            pb += nb
        planned.append((b0, nblk, parts))
    calls = planned

    # --- per-core cell maps ----------------------------------------------
    cell_srcs, cell_dsts = [], []
    for c in range(N_CORES):
        perm = perms[c]
        real = perm >= 0
        pos_of_dst = np.empty(DPC, np.int64)
        pos_of_dst[perm[real]] = np.flatnonzero(real)

        lo, hi = starts_all[c * DPC], starts_all[(c + 1) * DPC]
        e_src = src_s[lo:hi].astype(np.int64)
        e_dstl = dst_s[lo:hi] - c * DPC
        e_k = k_s[lo:hi]
        pos = pos_of_dst[e_dstl]
        e_t = pos // 128
        e_q = pos % 128
        tarr = np.array(tiles, np.int64)  # (cap, G, nblk, bstart)
        e_cap = tarr[e_t, 0]
        e_G = tarr[e_t, 1]
        e_bs = tarr[e_t, 3]
        e_kb = e_q // e_G
        e_g = e_q - e_kb * e_G
        e_p = (e_g * e_cap + e_k).astype(np.int64)
        e_b = (e_bs + e_kb).astype(np.int64)
        S = np.full((128, tot_blk), -1, np.int32)
        Dst = np.full((128, tot_blk), -1, np.int32)
        S[e_p, e_b] = e_src
        Dst[e_p, e_b] = c * DPC + e_dstl
        cell_srcs.append(S)
        cell_dsts.append(Dst)

    return Schedule(
        n_tiles, tot_blk, tiles, blocks, calls, caps, ones_cols,
        perms, cell_srcs, cell_dsts,
    )


# ---------------------------------------------------------------- prog A
def build_progA(n_loc=DPC, in_c=IN_C, f=F):
    # one fused matmul per 512-col chunk: lhsT = [W1 | W1@A1cat] so Wh and
    # both attention scalars come out of a single PSUM tile / copy
    nc = bacc.Bacc("TRN2", target_bir_lowering=False, debug=False, num_devices=N_CORES)
    xT = nc.dram_tensor("xT", [in_c, n_loc], F16, kind="ExternalInput").ap()
    WP = nc.dram_tensor("WP", [in_c, f + 2], F16, kind="ExternalInput").ap()
    bP = nc.dram_tensor("bP", [f + 2, 1], F32, kind="ExternalInput").ap()
    combA = nc.dram_tensor("combA", [f + 2, n_loc], F16, kind="ExternalOutput").ap()

    with tile.TileContext(nc) as tc:
        with tc.tile_pool(name="sb", bufs=1) as pool, tc.tile_pool(
            name="ps", bufs=4, space="PSUM"
        ) as pps, tc.tile_pool(name="sb2", bufs=3) as pool2:
            WP_sb = pool.tile([in_c, f + 2], F16)
            nc.sync.dma_start(out=WP_sb[:], in_=WP[:, :])
            bP_sb = pool.tile([f + 2, 1], F32)
            nc.sync.dma_start(out=bP_sb[:], in_=bP[:, :])
            xT_sb = pool.tile([in_c, n_loc], F16)
            XCH = 1563
            for x0 in range(0, n_loc, XCH):
                xc = min(XCH, n_loc - x0)
                nc.sync.dma_start(
                    out=xT_sb[:, x0 : x0 + xc], in_=xT[:, x0 : x0 + xc]
                )

            CH = 512
            GRP = 4
            wh_g = None
            for c0 in range(0, n_loc, CH):
                ch = min(CH, n_loc - c0)
                gi = (c0 // CH) % GRP
                if gi == 0:
                    wh_g = pool2.tile([f + 2, GRP * CH], F16, tag="whg")
                ps_w = pps.tile([f + 2, CH], F32, space="PSUM")
                nc.tensor.matmul(
                    out=ps_w[:, :ch],
                    lhsT=WP_sb[:],
                    rhs=xT_sb[:, c0 : c0 + ch],
                    start=True,
                    stop=True,
                )
                if (c0 // CH) % 2 == 0:
                    # alternate the PSUM->SBUF copy between Act and DVE
                    nc.scalar.activation(
                        out=wh_g[:, gi * CH : gi * CH + ch],
                        in_=ps_w[:, :ch],
                        func=mybir.ActivationFunctionType.Identity,
                        bias=bP_sb[:],
                    )
                else:
                    nc.vector.tensor_scalar(
                        out=wh_g[:, gi * CH : gi * CH + ch],
                        in0=ps_w[:, :ch],
                        scalar1=bP_sb[:, 0:1],
                        scalar2=None,
                        op0=mybir.AluOpType.add,
                    )
                if gi == GRP - 1 or c0 + ch >= n_loc:
                    g0 = (c0 // CH // GRP) * GRP * CH
                    gl = c0 + ch - g0
                    nc.sync.dma_start(
                        out=combA[:, g0 : g0 + gl], in_=wh_g[:, :gl]
                    )
    nc.compile()
    return nc


# ---------------------------------------------------------------- prog B
def build_progB(sched: Schedule, f=F):
    NT = sched.n_tiles
    TB = sched.tot_blk
    nc = bacc.Bacc("TRN2", target_bir_lowering=False, debug=False, num_devices=N_CORES)
    tbl = nc.dram_tensor("tbl", [128, TB * (f + 1)], F16, kind="ExternalInput").ap()
    sjsi = nc.dram_tensor("sjsi", [128, 2 * TB], F32, kind="ExternalInput").ap()
    # rows 0-63 x cols 0-65 = [Wn | Wn@As]; col 66 = bias (66 rows)
    wp_d = nc.dram_tensor("wpack", [f + 2, f + 3], F32, kind="ExternalInput").ap()
    ones_d = nc.dram_tensor(
        "ones", [128, sched.ones_cols + f], F16, kind="ExternalInput"
    ).ap()
    shf_d = nc.dram_tensor("shiftv", [128, 1], F32, kind="ExternalInput").ap()
    comb = nc.dram_tensor("comb", [f + 2, NT * 128], F16, kind="ExternalOutput").ap()

    AF = mybir.ActivationFunctionType
    OP = mybir.AluOpType
    P = f + 1  # 65: per-block pitch

    def v(ap, dims, off=0):
        return dataclasses.replace(
            ap,
            ap=[list(ap.ap[0])] + [list(d) for d in dims],
            offset=ap.offset + off,
        )

    with tile.TileContext(nc) as tc:
        with tc.tile_pool(name="const", bufs=1) as pc, tc.tile_pool(
            name="gat", bufs=5
        ) as pg, tc.tile_pool(name="exw", bufs=4) as px, tc.tile_pool(
            name="work", bufs=5
        ) as pw, tc.tile_pool(name="ps", bufs=2, space="PSUM") as pps, tc.tile_pool(
            name="psb", bufs=3, space="PSUM"
        ) as ppsb, tc.tile_pool(name="ep", bufs=3) as pep:
            wp_sb = pc.tile([f + 2, f + 3], F32)
            WC_sb = pc.tile([f, f + 2], F16)
            bias_sb = wp_sb[:, f + 2 : f + 3]
            ones_sb = pc.tile([128, sched.ones_cols + f], F16)
            ones1_sb = pc.tile([1, f], F32)
            nc.vector.memset(ones1_sb[:], 1.0)
            shf_sb = pc.tile([128, 1], F32)
            nc.sync.dma_start(out=shf_sb[:], in_=shf_d[:, :])
            nc.sync.dma_start(out=ones_sb[:], in_=ones_d[:, :])

            def emit_consts():
                nc.sync.dma_start(out=wp_sb[:], in_=wp_d[:, :])
                nc.vector.tensor_scalar(
                    out=WC_sb[:],
                    in0=wp_sb[:f, : f + 2],
                    scalar1=1.0,
                    scalar2=None,
                    op0=OP.mult,
                )

            ps_ch = None

            def flush_chunk(ck, ntl, ps_ch):
                cols = ntl * 128
                # leaky rides the PSUM->SBUF copy; positive rden scales
                # commute with leaky-relu so normalize happens after
                hTL = pep.tile([f, CHT * 128], F16, tag="hTL")
                nc.scalar.activation(
                    out=hTL[:, :cols],
                    in_=ps_ch[0:f, :cols],
                    func=AF.Prelu,
                    alpha=ALPHA,
                )
                rdn = pep.tile([1, CHT * 128], F16, tag="rdn")
                with nc.allow_low_precision(
                    reason="rden in f16: SHIFT_TARGET=6 bounds den to "
                    "[7e-5, 1.5e4], all normal-range f16"
                ):
                    nc.vector.reciprocal(
                        out=rdn[:, :cols], in_=ps_ch[f : f + 1, :cols]
                    )
                ps_r = pps.tile([f, CHT * 128], F32, tag="psr", space="PSUM")
                nc.tensor.matmul(
                    out=ps_r[:, :cols],
                    lhsT=ones_sb[0:1, sched.ones_cols : sched.ones_cols + f],
                    rhs=rdn[:, :cols],
                    start=True,
                    stop=True,
                )
                ht = pep.tile([f, CHT * 128], F16, tag="ht")
                nc.vector.tensor_tensor(
                    out=ht[:, :cols],
                    in0=hTL[:, :cols],
                    in1=ps_r[:, :cols],
                    op=OP.mult,
                )
                ps_c = pps.tile([f + 2, CHT * 128], F32, tag="psc", space="PSUM")
                nc.tensor.matmul(
                    out=ps_c[:, :cols],
                    lhsT=WC_sb[:],
                    rhs=ht[:, :cols],
                    start=True,
                    stop=True,
                )
                c_sb = pep.tile([f + 2, CHT * 128], F16, tag="csb")
                nc.scalar.activation(
                    out=c_sb[:, :cols],
                    in_=ps_c[:, :cols],
                    func=AF.Identity,
                    bias=bias_sb,
                )
                nc.sync.dma_start(
                    out=comb[:, ck * CHT * 128 : ck * CHT * 128 + cols],
                    in_=c_sb[:, :cols],
                )

            def stage1(ci):
                b0, nblk, parts = sched.calls[ci]
                sj_sb = pw.tile([128, 2 * BMAX], F32, tag="sjsi")
                nc.sync.dma_start(
                    out=sj_sb[:, : 2 * nblk],
                    in_=sjsi[:, 2 * b0 : 2 * b0 + 2 * nblk],
                )
                rhs = pg.tile([128, BMAX * P], F16, tag="rhs")
                nc.sync.dma_start(
                    out=rhs[:, : nblk * P], in_=tbl[:, b0 * P : (b0 + nblk) * P]
                )
                epre = pw.tile([128, BMAX], F32, tag="epre")
                nc.vector.tensor_tensor(
                    out=epre[:, :nblk],
                    in0=sj_sb[:, :nblk],
                    in1=sj_sb[:, nblk : 2 * nblk],
                    op=OP.add,
                )
                e1 = pw.tile([128, BMAX], F32, tag="e1")
                nc.scalar.activation(
                    out=e1[:, :nblk], in_=epre[:, :nblk], func=AF.Prelu, alpha=ALPHA
                )
                # exp lands in the col-64 holes of the streamed cells
                nc.scalar.activation(
                    out=v(rhs[:], [(P, nblk)], off=f),
                    in_=e1[:, :nblk],
                    func=AF.Exp,
                    bias=shf_sb[:],
                )
                exw = None
                for (pb, nb, m_pool) in parts:
                    if not m_pool:
                        if exw is None:
                            exw = px.tile([128, BMAX * EXPW], F16, tag="exw")
                        nc.scalar.activation(
                            out=v(
                                exw[:], [(EXPW, nb), (1, EXPW)], off=pb * EXPW
                            ),
                            in_=v(e1[:], [(1, nb), (0, EXPW)], off=pb),
                            func=AF.Exp,
                            bias=shf_sb[:],
                        )
                return rhs, exw

            def stage2a(ci, rhs, exw):
                b0, nblk, parts = sched.calls[ci]
                for (pb, nb, m_pool) in parts:
                    if m_pool:
                        nc.gpsimd.tensor_tensor(
                            out=v(rhs[:], [(P, nb), (1, f)], off=pb * P),
                            in0=v(rhs[:], [(P, nb), (1, f)], off=pb * P),
                            in1=v(rhs[:], [(P, nb), (0, f)], off=pb * P + f),
                            op=OP.mult,
                        )
                for (pb, nb, m_pool) in parts:
                    if not m_pool:
                        for q in range(0, f, EXPW):
                            nc.vector.tensor_tensor(
                                out=v(rhs[:], [(P, nb), (1, EXPW)], off=pb * P + q),
                                in0=v(rhs[:], [(P, nb), (1, EXPW)], off=pb * P + q),
                                in1=v(
                                    exw[:], [(EXPW, nb), (1, EXPW)], off=pb * EXPW
                                ),
                                op=OP.mult,
                            )

            def stage2b(ci, rhs, exw):
                nonlocal ps_ch
                b0, nblk, parts = sched.calls[ci]
                done = []
                for bi in range(nblk):
                    (t, kb, gg, dcol, gcol0) = sched.blocks[b0 + bi]
                    if t % CHT == 0 and kb == 0:
                        ps_ch = ppsb.tile(
                            [f + 1, CHT * 128], F32, tag="psch", space="PSUM"
                        )
                    nc.tensor.matmul(
                        out=ps_ch[:, dcol : dcol + gg],
                        lhsT=rhs[:, bi * P : (bi + 1) * P],
                        rhs=ones_sb[:, gcol0 : gcol0 + gg],
                        start=True,
                        stop=True,
                    )
                    last_of_tile = (
                        b0 + bi + 1 == TB
                        or sched.blocks[b0 + bi + 1][0] != t
                    )
                    if last_of_tile and (t % CHT == CHT - 1 or t == NT - 1):
                        done.append((t // CHT, t % CHT + 1, ps_ch))
                return done

            st1 = {}
            st2b = {}
            ncalls = len(sched.calls)
            for ci in range(ncalls + 4):
                if ci < ncalls:
                    st1[ci] = stage1(ci)
                    if ci == 0:
                        emit_consts()
                if 2 <= ci < ncalls + 2:
                    stage2a(ci - 2, *st1[ci - 2])
                if 3 <= ci < ncalls + 3:
                    st2b[ci - 3] = stage2b(ci - 3, *st1[ci - 3])
                if ci >= 4:
                    # flushes deferred one step so recip/norm never block
                    # the DVE stream while the chunk's matmuls still run
                    for (ck, ntl, ps) in st2b[ci - 4]:
                        flush_chunk(ck, ntl, ps)
    nc.compile()
    return nc


# ---------------------------------------------------------------- driver
_cache = {}
TRACE = False
LAST_HW_NS = []
LAST_RESULTS = []


def _run(nc, in_maps, cores):
    res = bass_utils.run_bass_kernel_spmd(nc, in_maps, core_ids=cores, trace=TRACE)
    if TRACE:
        LAST_RESULTS.append(res)
        if res.exec_time_ns:
            LAST_HW_NS.append(res.exec_time_ns)
    return res


def _get_schedule(edge_index):
    fp = hashlib.sha1(np.ascontiguousarray(edge_index)).hexdigest()
    key = ("sched", fp)
    if key not in _cache:
        _cache[key] = build_schedule(edge_index)
    return _cache[key]


def kernel(x, edge_index, W1, bW1, A1, bA1, W2, bW2, A2, bA2, Wfc, bfc):
    x = np.asarray(x, dtype=np.float32)
    edge_index = np.asarray(edge_index)
    W1 = np.asarray(W1, np.float32)
    bW1 = np.asarray(bW1, np.float32)
    A1 = np.asarray(A1, np.float32)
    bA1 = np.asarray(bA1, np.float32)
    W2 = np.asarray(W2, np.float32)
    bW2 = np.asarray(bW2, np.float32)
    A2 = np.asarray(A2, np.float32)
    bA2 = np.asarray(bA2, np.float32)
    Wfc = np.asarray(Wfc, np.float32)
    bfc = np.asarray(bfc, np.float32)

    sched = _get_schedule(edge_index)
    cores = list(range(N_CORES))

    if "A" not in _cache:
        _cache["A"] = build_progA()
    ncA = _cache["A"]
    inA = []
    x16 = x.astype(np.float16)
    A1cat = np.concatenate([A1[:F], A1[F:]], axis=1)  # [64, 2]
    WP = np.concatenate([W1, W1 @ A1cat], axis=1).astype(np.float16)  # [128, 66]
    bP = np.concatenate([bW1, bW1 @ A1cat]).reshape(F + 2, 1).astype(np.float32)
    for c in cores:
        xT = np.ascontiguousarray(x16[c * DPC : (c + 1) * DPC].T)
        inA.append({"xT": xT, "WP": WP, "bP": bP})
    LAST_HW_NS.clear()
    LAST_RESULTS.clear()
    resA = _run(ncA, inA, cores)
    cA = np.concatenate([resA.results[c]["combA"] for c in cores], axis=1)
    wh16 = np.ascontiguousarray(cA[:F].T)  # [N, 64] f16
    si_full = cA[F].astype(np.float32)
    sj_full = cA[F + 1].astype(np.float32)

    key = ("B", sched.n_tiles, sched.tot_blk, tuple(s[0] for s in sched.tiles))
    if key not in _cache:
        _cache[key] = build_progB(sched)
    ncB = _cache[key]
    NT = sched.n_tiles
    TB = sched.tot_blk

    ones_host = np.zeros((128, sched.ones_cols + F), np.float16)
    for (cap, G, gcol0) in sched.caps:
        for g in range(G):
            ones_host[g * cap : (g + 1) * cap, gcol0 + g] = 1
    ones_host[0, sched.ones_cols :] = 1  # rden partition-broadcast columns

    def launch_B(wh16_full, si_f, sj_f, bA, Wn, bWn, An):
        bA0 = np.float32(bA.reshape(-1)[0])
        WnAs = Wn @ An  # [64, 2]
        wpack = np.zeros((F + 2, F + 3), np.float32)
        wpack[:F, :F] = Wn
        wpack[:F, F : F + 2] = WnAs
        wpack[:F, F + 2] = bWn.reshape(F)
        wpack[F : F + 2, F + 2] = bWn @ An
        inB = []
        shift = np.float32(
            max(0.0, float(si_f.max() + sj_f.max() + bA0) - SHIFT_TARGET)
        )
        for c in cores:
            ss = sched.cell_src[c]
            dd = sched.cell_dst[c]
            m = ss >= 0
            t = np.zeros((128, TB, F + 1), np.float16)
            t[:, :, :F] = wh16_full[np.clip(ss, 0, N_NODES - 1)]
            t[:, :, :F][~m] = 0
            sj_cell = np.full((128, TB), np.float32(PAD_SJ))
            sj_cell[m] = sj_f[ss[m]]
            si_cell = np.zeros((128, TB), np.float32)
            si_cell[m] = si_f[dd[m]] + bA0
            sjsi = np.empty((128, 2 * TB), np.float32)
            for (b0, nblk, _) in sched.calls:
                sjsi[:, 2 * b0 : 2 * b0 + nblk] = sj_cell[:, b0 : b0 + nblk]
                sjsi[:, 2 * b0 + nblk : 2 * (b0 + nblk)] = si_cell[
                    :, b0 : b0 + nblk
                ]
            inB.append(
                {
                    "tbl": np.ascontiguousarray(t.reshape(128, TB * (F + 1))),
                    "sjsi": sjsi,
                    "wpack": wpack,
                    "ones": ones_host,
                    "shiftv": np.full((128, 1), -shift, np.float32),
                }
            )
        res = _run(ncB, inB, cores)
        whn = np.zeros((N_NODES, F), np.float16)
        sn_i = np.zeros(N_NODES, np.float32)
        sn_j = np.zeros(N_NODES, np.float32)
        for c in cores:
            perm = sched.perms[c]
            real = perm >= 0
            gids = c * DPC + perm[real]
            cb = res.results[c]["comb"]
            whn[gids] = cb[:F].T[real]
            sn_i[gids] = cb[F].astype(np.float32)[real]
            sn_j[gids] = cb[F + 1].astype(np.float32)[real]
        return whn, sn_i, sn_j

    As2 = np.ascontiguousarray(np.concatenate([A2[:F], A2[F:]], axis=1))
    wh2, si2, sj2 = launch_B(wh16, si_full, sj_full, bA1, W2, bW2, As2)
    out, _, _ = launch_B(wh2, si2, sj2, bA2, Wfc, bfc, np.zeros((F, 2), np.float32))
    return out.astype(np.float32)


# revision 50
# speedup vs baseline: 1.5177x; 1.0332x over previous
"""GAT (2-layer) on 8 NeuronCores — Bass/Tile kernel.

Strategy (dst-sharded graph parallel, PE segment-sum):
  - Each core owns 12500 destination nodes, degree-sorted descending into
    98 tiles of 128 dsts. Per-tile slot capacity c = a near-exact cap on
    the tile's max degree chosen so G = 128//c destination nodes pack one
    128-partition block (partition p = dst g=p//c, slot s=p%c).
  - The halo "gather" is a plain 2D stream: the host packs each core's
    cells in exact consumption order ([128, blocks x 65 fp16]; 64 feature
    cols per block plus a hole at col 64 for exp), so the kernel issues
    one contiguous dma_start per call — no dma_gather, no index tables.
  - The segment softmax sum runs on the Tensor engine: per block, one
    matmul with lhsT = the 65-col data block and rhs = a constant
    block-diagonal ones matrix produces out[65, G] — feature-major
    (already transposed for the epilogue) with the softmax denominator
    riding as row 64. Disjoint psum columns per block; no tree, no
    accumulation hazards.
  - Per-edge exp weights: e1 = leaky(sj+si) (Act), exp written into the
    col-64 holes (Act), then the feature multiply is split between
    [Act 32-wide exp broadcast-expand + DVE 2x-rate fp16 multiply] and a
    fused stride-0 broadcast multiply on Pool (gpsimd), balanced by a
    static per-call planner.
  - Normalization: rden = recip(den row), partition-broadcast by a
    1-partition ones matmul into PSUM, multiplied after the leaky
    PSUM->SBUF copy (positive scales commute with leaky-relu). The
    epilogue is one fused matmul [Wn | Wn@As] producing whn and the next
    layer's attention scalars in a single fp16 output tensor.
  - Launch A: one fused matmul per chunk with lhsT = [W1 | W1@A1cat].
  - Host between launches does index/layout work only (cell packing from
    device-computed Wh, sj/si expansion, weight folding); all value
    FLOPs run on device.
"""

import dataclasses
import hashlib
import numpy as np

import concourse.bacc as bacc
import concourse.tile as tile
from concourse import bass, mybir, bass_utils

F32 = mybir.dt.float32
F16 = mybir.dt.float16

N_NODES = 100000
N_CORES = 8
DPC = N_NODES // N_CORES
F = 64
IN_C = 128
BMAX = 160  # blocks per call
EXPW = 32  # exp expansion width; DVE multiplies in F/EXPW passes
CHT = 4  # tiles per epilogue chunk (512 dsts)
PAD_SJ = -60.0
ALPHA = 0.2
SHIFT_TARGET = 6.0  # keep exp<=e^6 so f16 denominators can't overflow

# planner cost constants (ns); see TRN2Spec in concourse/hw_specs.py
ACT_EL = 0.833
DVE2X = 0.5208
DVE1X = 1.0417
POOL_EL = 1.984  # 0.833 / 0.42 gpsimd Add/Multiply efficiency
FIX_ACT = 185.0
FIX_DVE = 146.0
FIX_POOL = 60.0


@dataclasses.dataclass
class Schedule:
    n_tiles: int
    tot_blk: int
    chunk_cols: list  # per psum chunk: live dst columns
    blocks: list  # per block: (chunk, gg, dcol, gcol0)
    calls: list  # (b0, nblk, parts) ; parts: [(pb0, nb, m_pool)]
    caps: list  # [(cap, G, gcol0)]
    ones_cols: int
    perms: list  # per core: int64 [n_tiles*128], local dst or -1
    cell_src: list  # per core: int32 [128, tot_blk], global src or -1
    cell_dst: list  # per core: int32 [128, tot_blk], global dst or -1


def _best_cap(D):
    best = None
    for c in range(max(2, D), 129):
        G = 128 // c
        if G == 0:
            break
        ov = (128.0 / G) / D
        if best is None or ov < best[0]:
            best = (ov, c, G)
    return best[1], best[2]


def build_schedule(edge_index: np.ndarray) -> Schedule:
    src = np.asarray(edge_index[0], dtype=np.int64)
    dst = np.asarray(edge_index[1], dtype=np.int64)
    E = src.shape[0]
    order = np.argsort(dst, kind="stable")
    src_s = src[order]
    dst_s = dst[order]
    deg_all = np.bincount(dst, minlength=N_NODES)
    starts_all = np.concatenate([[0], np.cumsum(deg_all)])
    k_s = np.arange(E) - starts_all[dst_s]

    nt = -(-DPC // 128)
    perms = []
    tmax = np.zeros(nt)
    for c in range(N_CORES):
        deg = deg_all[c * DPC : (c + 1) * DPC]
        rank = np.argsort(deg, kind="stable")[::-1]
        perm = np.full(nt * 128, -1, dtype=np.int64)
        perm[:DPC] = rank
        perms.append(perm)
        d = np.where(perm >= 0, deg[np.clip(perm, 0, DPC - 1)], 0)
        tmax = np.maximum(tmax, d.reshape(nt, 128).max(1))
    n_tiles = nt

    # per-position max degree across cores (non-increasing: each core is
    # degree-desc sorted)
    npos = n_tiles * 128
    dpos = np.zeros(npos, np.int64)
    for c in range(N_CORES):
        deg = deg_all[c * DPC : (c + 1) * DPC]
        dp = np.zeros(npos, np.int64)
        dp[:DPC] = np.sort(deg)[::-1]
        dpos = np.maximum(dpos, dp)

    # greedy variable-cap blocks: cap = degree of the first (largest) dst of
    # the block; blocks are free to cross tile boundaries but not PSUM
    # chunk boundaries
    CHP = CHT * 128
    blk_of_pos = np.zeros(npos, np.int64)
    g_of_pos = np.zeros(npos, np.int64)
    cap_of_pos = np.zeros(npos, np.int64)
    blocks = []  # (ck, gg, dcol, cap)
    caps_used = {}
    q = 0
    while q < npos:
        chunk_end = min((q // CHP + 1) * CHP, npos)
        cap = max(1, int(dpos[q]))
        G = 128 // cap
        gg = min(G, chunk_end - q)
        caps_used.setdefault(cap, G)
        b = len(blocks)
        blocks.append((q // CHP, gg, q % CHP, cap))
        blk_of_pos[q : q + gg] = b
        g_of_pos[q : q + gg] = np.arange(gg)
        cap_of_pos[q : q + gg] = cap
        q += gg
    tot_blk = len(blocks)
    n_chunks = -(-npos // CHP)
    chunk_cols = [min(CHP, npos - ck * CHP) for ck in range(n_chunks)]

    caps = []
    gcol = 0
    capmap = {}
    for cap in sorted(caps_used):
        G = caps_used[cap]
        caps.append((cap, G, gcol))
        capmap[cap] = gcol
        gcol += G
    ones_cols = gcol
    blocks = [(ck, gg, dcol, capmap[cap]) for (ck, gg, dcol, cap) in blocks]

    # pack block runs into calls of <= BMAX blocks (ramped starts)
    calls = []
    b0 = 0
    while b0 < tot_blk:
        lim = (24, 48)[len(calls)] if len(calls) < 2 else BMAX
        nblk = min(lim, tot_blk - b0)
        calls.append((b0, nblk, None))
        b0 += nblk

    # --- static engine planner: split the multiply DVE vs Pool per call ---
    planned = []
    flushA = 2 * (FIX_ACT + 512 * ACT_EL) / CHT
    flushD = (FIX_DVE + 512 * DVE1X + FIX_DVE + 512 * DVE2X) / CHT
    ncalls_total = len(calls)
    for cidx, (b0, nblk, _) in enumerate(calls):
        pool_el = POOL_EL * (1.7 if cidx >= ncalls_total - 2 else 1.0)
        ntl = 4 * sum(
            1
            for bi in range(b0, b0 + nblk)
            if bi + 1 == tot_blk or blocks[bi + 1][0] != blocks[bi][0]
        )
        actT = 2 * FIX_ACT + 2 * nblk * ACT_EL + ntl * flushA  # e1+exp+flush
        dveT = FIX_DVE + nblk * DVE1X + ntl * flushD  # epre + flush
        poolT = 0.0
        parts = []
        pb = 0
        while pb < nblk:
            nb = min(12, nblk - pb)
            a_x = actT + FIX_ACT + nb * EXPW * ACT_EL
            d_x = dveT + (64 // EXPW) * FIX_DVE + nb * 64 * DVE2X
            p_y = poolT + FIX_POOL + nb * 64 * pool_el
            if max(a_x, d_x, poolT) <= max(actT, dveT, p_y):
                actT, dveT = a_x, d_x
                parts.append((pb, nb, 0))
            else:
                poolT = p_y
                parts.append((pb, nb, 1))
            pb += nb
        planned.append((b0, nblk, parts))
    calls = planned

    # --- per-core cell maps ----------------------------------------------
    cell_srcs, cell_dsts = [], []
    for c in range(N_CORES):
        perm = perms[c]
        real = perm >= 0
        pos_of_dst = np.empty(DPC, np.int64)
        pos_of_dst[perm[real]] = np.flatnonzero(real)

        lo, hi = starts_all[c * DPC], starts_all[(c + 1) * DPC]
        e_src = src_s[lo:hi].astype(np.int64)
        e_dstl = dst_s[lo:hi] - c * DPC
        e_k = k_s[lo:hi]
        pos = pos_of_dst[e_dstl]
        e_p = (g_of_pos[pos] * cap_of_pos[pos] + e_k).astype(np.int64)
        e_b = blk_of_pos[pos]
        S = np.full((128, tot_blk), -1, np.int32)
        Dst = np.full((128, tot_blk), -1, np.int32)
        S[e_p, e_b] = e_src
        Dst[e_p, e_b] = c * DPC + e_dstl
        cell_srcs.append(S)
        cell_dsts.append(Dst)

    return Schedule(
        n_tiles, tot_blk, chunk_cols, blocks, calls, caps, ones_cols,
        perms, cell_srcs, cell_dsts,
    )


# ---------------------------------------------------------------- prog A
def build_progA(n_loc=DPC, in_c=IN_C, f=F):
    # one fused matmul per 512-col chunk: lhsT = [W1 | W1@A1cat] so Wh and
    # both attention scalars come out of a single PSUM tile / copy
    nc = bacc.Bacc("TRN2", target_bir_lowering=False, debug=False, num_devices=N_CORES)
    xT = nc.dram_tensor("xT", [in_c, n_loc], F16, kind="ExternalInput").ap()
    WP = nc.dram_tensor("WP", [in_c, f + 2], F16, kind="ExternalInput").ap()
    bP = nc.dram_tensor("bP", [f + 2, 1], F32, kind="ExternalInput").ap()
    combA = nc.dram_tensor("combA", [f + 2, n_loc], F16, kind="ExternalOutput").ap()

    with tile.TileContext(nc) as tc:
        with tc.tile_pool(name="sb", bufs=1) as pool, tc.tile_pool(
            name="ps", bufs=4, space="PSUM"
        ) as pps, tc.tile_pool(name="sb2", bufs=3) as pool2:
            WP_sb = pool.tile([in_c, f + 2], F16)
            nc.sync.dma_start(out=WP_sb[:], in_=WP[:, :])
            bP_sb = pool.tile([f + 2, 1], F32)
            nc.sync.dma_start(out=bP_sb[:], in_=bP[:, :])
            xT_sb = pool.tile([in_c, n_loc], F16)
            XCH = 1563
            for x0 in range(0, n_loc, XCH):
                xc = min(XCH, n_loc - x0)
                nc.sync.dma_start(
                    out=xT_sb[:, x0 : x0 + xc], in_=xT[:, x0 : x0 + xc]
                )

            CH = 512
            GRP = 4
            wh_g = None
            for c0 in range(0, n_loc, CH):
                ch = min(CH, n_loc - c0)
                gi = (c0 // CH) % GRP
                if gi == 0:
                    wh_g = pool2.tile([f + 2, GRP * CH], F16, tag="whg")
                ps_w = pps.tile([f + 2, CH], F32, space="PSUM")
                nc.tensor.matmul(
                    out=ps_w[:, :ch],
                    lhsT=WP_sb[:],
                    rhs=xT_sb[:, c0 : c0 + ch],
                    start=True,
                    stop=True,
                )
                if (c0 // CH) % 2 == 0:
                    # alternate the PSUM->SBUF copy between Act and DVE
                    nc.scalar.activation(
                        out=wh_g[:, gi * CH : gi * CH + ch],
                        in_=ps_w[:, :ch],
                        func=mybir.ActivationFunctionType.Identity,
                        bias=bP_sb[:],
                    )
                else:
                    nc.vector.tensor_scalar(
                        out=wh_g[:, gi * CH : gi * CH + ch],
                        in0=ps_w[:, :ch],
                        scalar1=bP_sb[:, 0:1],
                        scalar2=None,
                        op0=mybir.AluOpType.add,
                    )
                if gi == GRP - 1 or c0 + ch >= n_loc:
                    g0 = (c0 // CH // GRP) * GRP * CH
                    gl = c0 + ch - g0
                    nc.sync.dma_start(
                        out=combA[:, g0 : g0 + gl], in_=wh_g[:, :gl]
                    )
    nc.compile()
    return nc


# ---------------------------------------------------------------- prog B
def build_progB(sched: Schedule, f=F):
    NT = sched.n_tiles
    TB = sched.tot_blk
    nc = bacc.Bacc("TRN2", target_bir_lowering=False, debug=False, num_devices=N_CORES)
    tbl = nc.dram_tensor("tbl", [128, TB * (f + 1)], F16, kind="ExternalInput").ap()
    sjsi = nc.dram_tensor("sjsi", [128, 2 * TB], F16, kind="ExternalInput").ap()
    # rows 0-63 x cols 0-65 = [Wn | Wn@As]; col 66 = bias (66 rows)
    wp_d = nc.dram_tensor("wpack", [f + 2, f + 3], F32, kind="ExternalInput").ap()
    ones_d = nc.dram_tensor(
        "ones", [128, sched.ones_cols + f], F16, kind="ExternalInput"
    ).ap()
    shf_d = nc.dram_tensor("shiftv", [128, 1], F32, kind="ExternalInput").ap()
    comb = nc.dram_tensor("comb", [f + 2, NT * 128], F16, kind="ExternalOutput").ap()

    AF = mybir.ActivationFunctionType
    OP = mybir.AluOpType
    P = f + 1  # 65: per-block pitch

    def v(ap, dims, off=0):
        return dataclasses.replace(
            ap,
            ap=[list(ap.ap[0])] + [list(d) for d in dims],
            offset=ap.offset + off,
        )

    with tile.TileContext(nc) as tc:
        with tc.tile_pool(name="const", bufs=1) as pc, tc.tile_pool(
            name="gat", bufs=5
        ) as pg, tc.tile_pool(name="exw", bufs=4) as px, tc.tile_pool(
            name="work", bufs=5
        ) as pw, tc.tile_pool(name="ps", bufs=2, space="PSUM") as pps, tc.tile_pool(
            name="psb", bufs=3, space="PSUM"
        ) as ppsb, tc.tile_pool(name="ep", bufs=3) as pep:
            wp_sb = pc.tile([f + 2, f + 3], F32)
            WC_sb = pc.tile([f, f + 2], F16)
            bias_sb = wp_sb[:, f + 2 : f + 3]
            ones_sb = pc.tile([128, sched.ones_cols + f], F16)
            ones1_sb = pc.tile([1, f], F32)
            nc.vector.memset(ones1_sb[:], 1.0)
            shf_sb = pc.tile([128, 1], F32)
            nc.sync.dma_start(out=shf_sb[:], in_=shf_d[:, :])
            nc.sync.dma_start(out=ones_sb[:], in_=ones_d[:, :])

            def emit_consts():
                nc.sync.dma_start(out=wp_sb[:], in_=wp_d[:, :])
                nc.vector.tensor_scalar(
                    out=WC_sb[:],
                    in0=wp_sb[:f, : f + 2],
                    scalar1=1.0,
                    scalar2=None,
                    op0=OP.mult,
                )

            ps_ch = None

            def flush_chunk(ck, cols, ps_ch):
                # leaky rides the PSUM->SBUF copy; positive rden scales
                # commute with leaky-relu so normalize happens after
                hTL = pep.tile([f, CHT * 128], F16, tag="hTL")
                nc.scalar.activation(
                    out=hTL[:, :cols],
                    in_=ps_ch[0:f, :cols],
                    func=AF.Prelu,
                    alpha=ALPHA,
                )
                rdn = pep.tile([1, CHT * 128], F16, tag="rdn")
                with nc.allow_low_precision(
                    reason="rden in f16: SHIFT_TARGET=6 bounds den to "
                    "[7e-5, 1.5e4], all normal-range f16"
                ):
                    nc.vector.reciprocal(
                        out=rdn[:, :cols], in_=ps_ch[f : f + 1, :cols]
                    )
                ps_r = pps.tile([f, CHT * 128], F32, tag="psr", space="PSUM")
                nc.tensor.matmul(
                    out=ps_r[:, :cols],
                    lhsT=ones_sb[0:1, sched.ones_cols : sched.ones_cols + f],
                    rhs=rdn[:, :cols],
                    start=True,
                    stop=True,
                )
                ht = pep.tile([f, CHT * 128], F16, tag="ht")
                nc.vector.tensor_tensor(
                    out=ht[:, :cols],
                    in0=hTL[:, :cols],
                    in1=ps_r[:, :cols],
                    op=OP.mult,
                )
                ps_c = pps.tile([f + 2, CHT * 128], F32, tag="psc", space="PSUM")
                nc.tensor.matmul(
                    out=ps_c[:, :cols],
                    lhsT=WC_sb[:],
                    rhs=ht[:, :cols],
                    start=True,
                    stop=True,
                )
                c_sb = pep.tile([f + 2, CHT * 128], F16, tag="csb")
                nc.scalar.activation(
                    out=c_sb[:, :cols],
                    in_=ps_c[:, :cols],
                    func=AF.Identity,
                    bias=bias_sb,
                )
                nc.sync.dma_start(
                    out=comb[:, ck * CHT * 128 : ck * CHT * 128 + cols],
                    in_=c_sb[:, :cols],
                )

            def stage1(ci):
                b0, nblk, parts = sched.calls[ci]
                sj_sb = pw.tile([128, 2 * BMAX], F16, tag="sjsi")
                nc.sync.dma_start(
                    out=sj_sb[:, : 2 * nblk],
                    in_=sjsi[:, 2 * b0 : 2 * b0 + 2 * nblk],
                )
                rhs = pg.tile([128, BMAX * P], F16, tag="rhs")
                nc.sync.dma_start(
                    out=rhs[:, : nblk * P], in_=tbl[:, b0 * P : (b0 + nblk) * P]
                )
                epre = pw.tile([128, BMAX], F32, tag="epre")
                nc.vector.tensor_tensor(
                    out=epre[:, :nblk],
                    in0=sj_sb[:, :nblk],
                    in1=sj_sb[:, nblk : 2 * nblk],
                    op=OP.add,
                )
                e1 = pw.tile([128, BMAX], F32, tag="e1")
                nc.scalar.activation(
                    out=e1[:, :nblk], in_=epre[:, :nblk], func=AF.Prelu, alpha=ALPHA
                )
                # exp lands in the col-64 holes of the streamed cells
                nc.scalar.activation(
                    out=v(rhs[:], [(P, nblk)], off=f),
                    in_=e1[:, :nblk],
                    func=AF.Exp,
                    bias=shf_sb[:],
                )
                exw = None
                for (pb, nb, m_pool) in parts:
                    if not m_pool:
                        if exw is None:
                            exw = px.tile([128, BMAX * EXPW], F16, tag="exw")
                        nc.scalar.activation(
                            out=v(
                                exw[:], [(EXPW, nb), (1, EXPW)], off=pb * EXPW
                            ),
                            in_=v(e1[:], [(1, nb), (0, EXPW)], off=pb),
                            func=AF.Exp,
                            bias=shf_sb[:],
                        )
                return rhs, exw

            def stage2a(ci, rhs, exw):
                b0, nblk, parts = sched.calls[ci]
                for (pb, nb, m_pool) in parts:
                    if m_pool:
                        nc.gpsimd.tensor_tensor(
                            out=v(rhs[:], [(P, nb), (1, f)], off=pb * P),
                            in0=v(rhs[:], [(P, nb), (1, f)], off=pb * P),
                            in1=v(rhs[:], [(P, nb), (0, f)], off=pb * P + f),
                            op=OP.mult,
                        )
                for (pb, nb, m_pool) in parts:
                    if not m_pool:
                        for q in range(0, f, EXPW):
                            nc.vector.tensor_tensor(
                                out=v(rhs[:], [(P, nb), (1, EXPW)], off=pb * P + q),
                                in0=v(rhs[:], [(P, nb), (1, EXPW)], off=pb * P + q),
                                in1=v(
                                    exw[:], [(EXPW, nb), (1, EXPW)], off=pb * EXPW
                                ),
                                op=OP.mult,
                            )

            def stage2b(ci, rhs, exw):
                nonlocal ps_ch
                b0, nblk, parts = sched.calls[ci]
                done = []
                for bi in range(nblk):
                    (ck, gg, dcol, gcol0) = sched.blocks[b0 + bi]
                    if dcol == 0:
                        ps_ch = ppsb.tile(
                            [f + 1, CHT * 128], F32, tag="psch", space="PSUM"
                        )
                    nc.tensor.matmul(
                        out=ps_ch[:, dcol : dcol + gg],
                        lhsT=rhs[:, bi * P : (bi + 1) * P],
                        rhs=ones_sb[:, gcol0 : gcol0 + gg],
                        start=True,
                        stop=True,
                    )
                    if b0 + bi + 1 == TB or sched.blocks[b0 + bi + 1][0] != ck:
                        done.append((ck, sched.chunk_cols[ck], ps_ch))
                return done

            st1 = {}
            st2b = {}
            ncalls = len(sched.calls)
            for ci in range(ncalls + 4):
                if ci < ncalls:
                    st1[ci] = stage1(ci)
                    if ci == 0:
                        emit_consts()
                if 2 <= ci < ncalls + 2:
                    stage2a(ci - 2, *st1[ci - 2])
                if 3 <= ci < ncalls + 3:
                    st2b[ci - 3] = stage2b(ci - 3, *st1[ci - 3])
                if ci >= 4:
                    # flushes deferred one step so recip/norm never block
                    # the DVE stream while the chunk's matmuls still run
                    for (ck, cols, ps) in st2b[ci - 4]:
                        flush_chunk(ck, cols, ps)
    nc.compile()
    return nc


# ---------------------------------------------------------------- driver
_cache = {}
TRACE = False
LAST_HW_NS = []
LAST_RESULTS = []


def _run(nc, in_maps, cores):
    res = bass_utils.run_bass_kernel_spmd(nc, in_maps, core_ids=cores, trace=TRACE)
    if TRACE:
        LAST_RESULTS.append(res)
        if res.exec_time_ns:
            LAST_HW_NS.append(res.exec_time_ns)
    return res


def _get_schedule(edge_index):
    fp = hashlib.sha1(np.ascontiguousarray(edge_index)).hexdigest()
    key = ("sched", fp)
    if key not in _cache:
        _cache[key] = build_schedule(edge_index)
    return _cache[key]


def kernel(x, edge_index, W1, bW1, A1, bA1, W2, bW2, A2, bA2, Wfc, bfc):
    x = np.asarray(x, dtype=np.float32)
    edge_index = np.asarray(edge_index)
    W1 = np.asarray(W1, np.float32)
    bW1 = np.asarray(bW1, np.float32)
    A1 = np.asarray(A1, np.float32)
    bA1 = np.asarray(bA1, np.float32)
    W2 = np.asarray(W2, np.float32)
    bW2 = np.asarray(bW2, np.float32)
    A2 = np.asarray(A2, np.float32)
    bA2 = np.asarray(bA2, np.float32)
    Wfc = np.asarray(Wfc, np.float32)
    bfc = np.asarray(bfc, np.float32)

    sched = _get_schedule(edge_index)
    cores = list(range(N_CORES))

    if "A" not in _cache:
        _cache["A"] = build_progA()
    ncA = _cache["A"]
    inA = []
    x16 = x.astype(np.float16)
    A1cat = np.concatenate([A1[:F], A1[F:]], axis=1)  # [64, 2]
    WP = np.concatenate([W1, W1 @ A1cat], axis=1).astype(np.float16)  # [128, 66]
    bP = np.concatenate([bW1, bW1 @ A1cat]).reshape(F + 2, 1).astype(np.float32)
    for c in cores:
        xT = np.ascontiguousarray(x16[c * DPC : (c + 1) * DPC].T)
        inA.append({"xT": xT, "WP": WP, "bP": bP})
    LAST_HW_NS.clear()
    LAST_RESULTS.clear()
    resA = _run(ncA, inA, cores)
    cA = np.concatenate([resA.results[c]["combA"] for c in cores], axis=1)
    wh16 = np.ascontiguousarray(cA[:F].T)  # [N, 64] f16
    si_full = cA[F].astype(np.float32)
    sj_full = cA[F + 1].astype(np.float32)

    key = ("B", sched.n_tiles, sched.tot_blk, tuple(s[0] for s in sched.caps))
    if key not in _cache:
        _cache[key] = build_progB(sched)
    ncB = _cache[key]
    NT = sched.n_tiles
    TB = sched.tot_blk

    ones_host = np.zeros((128, sched.ones_cols + F), np.float16)
    for (cap, G, gcol0) in sched.caps:
        for g in range(G):
            ones_host[g * cap : (g + 1) * cap, gcol0 + g] = 1
    ones_host[0, sched.ones_cols :] = 1  # rden partition-broadcast columns

    def launch_B(wh16_full, si_f, sj_f, bA, Wn, bWn, An):
        bA0 = np.float32(bA.reshape(-1)[0])
        WnAs = Wn @ An  # [64, 2]
        wpack = np.zeros((F + 2, F + 3), np.float32)
        wpack[:F, :F] = Wn
        wpack[:F, F : F + 2] = WnAs
        wpack[:F, F + 2] = bWn.reshape(F)
        wpack[F : F + 2, F + 2] = bWn @ An
        inB = []
        shift = np.float32(
            max(0.0, float(si_f.max() + sj_f.max() + bA0) - SHIFT_TARGET)
        )
        for c in cores:
            ss = sched.cell_src[c]
            dd = sched.cell_dst[c]
            m = ss >= 0
            t = np.zeros((128, TB, F + 1), np.float16)
            t[:, :, :F] = wh16_full[np.clip(ss, 0, N_NODES - 1)]
            t[:, :, :F][~m] = 0
            sj_cell = np.full((128, TB), np.float32(PAD_SJ))
            sj_cell[m] = sj_f[ss[m]]
            si_cell = np.zeros((128, TB), np.float32)
            si_cell[m] = si_f[dd[m]] + bA0
            sjsi = np.empty((128, 2 * TB), np.float16)
            for (b0, nblk, _) in sched.calls:
                sjsi[:, 2 * b0 : 2 * b0 + nblk] = sj_cell[:, b0 : b0 + nblk]
                sjsi[:, 2 * b0 + nblk : 2 * (b0 + nblk)] = si_cell[
                    :, b0 : b0 + nblk
                ]
            inB.append(
                {
                    "tbl": np.ascontiguousarray(t.reshape(128, TB * (F + 1))),
                    "sjsi": sjsi,
                    "wpack": wpack,
                    "ones": ones_host,
                    "shiftv": np.full((128, 1), -shift, np.float32),
                }
            )
        res = _run(ncB, inB, cores)
        whn = np.zeros((N_NODES, F), np.float16)
        sn_i = np.zeros(N_NODES, np.float32)
        sn_j = np.zeros(N_NODES, np.float32)
        for c in cores:
            perm = sched.perms[c]
            real = perm >= 0
            gids = c * DPC + perm[real]
            cb = res.results[c]["comb"]
            whn[gids] = cb[:F].T[real]
            sn_i[gids] = cb[F].astype(np.float32)[real]
            sn_j[gids] = cb[F + 1].astype(np.float32)[real]
        return whn, sn_i, sn_j

    As2 = np.ascontiguousarray(np.concatenate([A2[:F], A2[F:]], axis=1))
    wh2, si2, sj2 = launch_B(wh16, si_full, sj_full, bA1, W2, bW2, As2)
    out, _, _ = launch_B(wh2, si2, sj2, bA2, Wfc, bfc, np.zeros((F, 2), np.float32))
    return out.astype(np.float32)


# revision 51
# speedup vs baseline: 1.5720x; 1.0358x over previous
"""GAT (2-layer) on 8 NeuronCores — Bass/Tile kernel.

Strategy (dst-sharded graph parallel, PE segment-sum):
  - Each core owns 12500 destination nodes, degree-sorted descending into
    98 tiles of 128 dsts. Per-tile slot capacity c = a near-exact cap on
    the tile's max degree chosen so G = 128//c destination nodes pack one
    128-partition block (partition p = dst g=p//c, slot s=p%c).
  - The halo "gather" is a plain 2D stream: the host packs each core's
    cells in exact consumption order ([128, blocks x 65 fp16]; 64 feature
    cols per block plus a hole at col 64 for exp), so the kernel issues
    one contiguous dma_start per call — no dma_gather, no index tables.
  - The segment softmax sum runs on the Tensor engine: per block, one
    matmul with lhsT = the 65-col data block and rhs = a constant
    block-diagonal ones matrix produces out[65, G] — feature-major
    (already transposed for the epilogue) with the softmax denominator
    riding as row 64. Disjoint psum columns per block; no tree, no
    accumulation hazards.
  - Per-edge exp weights: e1 = leaky(sj+si) (Act), exp written into the
    col-64 holes (Act), then the feature multiply is split between
    [Act 32-wide exp broadcast-expand + DVE 2x-rate fp16 multiply] and a
    fused stride-0 broadcast multiply on Pool (gpsimd), balanced by a
    static per-call planner.
  - Normalization: rden = recip(den row), partition-broadcast by a
    1-partition ones matmul into PSUM, multiplied after the leaky
    PSUM->SBUF copy (positive scales commute with leaky-relu). The
    epilogue is one fused matmul [Wn | Wn@As] producing whn and the next
    layer's attention scalars in a single fp16 output tensor.
  - Launch A: one fused matmul per chunk with lhsT = [W1 | W1@A1cat].
  - Host between launches does index/layout work only (cell packing from
    device-computed Wh, sj/si expansion, weight folding); all value
    FLOPs run on device.
"""

import dataclasses
import hashlib
import numpy as np

import concourse.bacc as bacc
import concourse.tile as tile
from concourse import bass, mybir, bass_utils

F32 = mybir.dt.float32
F16 = mybir.dt.float16

N_NODES = 100000
N_CORES = 8
DPC = N_NODES // N_CORES
F = 64
IN_C = 128
BMAX = 160  # blocks per call
EXPW = 32  # exp expansion width; DVE multiplies in F/EXPW passes
CHT = 4  # tiles per epilogue chunk (512 dsts)
PAD_SJ = -60.0
ALPHA = 0.2
SHIFT_TARGET = 6.0  # keep exp<=e^6 so f16 denominators can't overflow

# planner cost constants (ns); see TRN2Spec in concourse/hw_specs.py
ACT_EL = 0.833
DVE2X = 0.5208
DVE1X = 1.0417
POOL_EL = 1.984  # 0.833 / 0.42 gpsimd Add/Multiply efficiency
FIX_ACT = 185.0
FIX_DVE = 146.0
FIX_POOL = 60.0


@dataclasses.dataclass
class Schedule:
    n_tiles: int
    tot_blk: int
    chunk_cols: list  # per psum chunk: live dst columns
    blocks: list  # per block: (chunk, gg, dcol, gcol0)
    calls: list  # (b0, nblk, parts) ; parts: [(pb0, nb, m_pool)]
    caps: list  # [(cap, G, gcol0)]
    ones_cols: int
    perms: list  # per core: int64 [n_tiles*128], local dst or -1
    cell_src: list  # per core: int32 [128, tot_blk], global src or -1
    cell_dst: list  # per core: int32 [128, tot_blk], global dst or -1


def _best_cap(D):
    best = None
    for c in range(max(2, D), 129):
        G = 128 // c
        if G == 0:
            break
        ov = (128.0 / G) / D
        if best is None or ov < best[0]:
            best = (ov, c, G)
    return best[1], best[2]


def build_schedule(edge_index: np.ndarray) -> Schedule:
    src = np.asarray(edge_index[0], dtype=np.int64)
    dst = np.asarray(edge_index[1], dtype=np.int64)
    E = src.shape[0]
    order = np.argsort(dst, kind="stable")
    src_s = src[order]
    dst_s = dst[order]
    deg_all = np.bincount(dst, minlength=N_NODES)
    starts_all = np.concatenate([[0], np.cumsum(deg_all)])
    k_s = np.arange(E) - starts_all[dst_s]

    nt = -(-DPC // 128)
    perms = []
    tmax = np.zeros(nt)
    for c in range(N_CORES):
        deg = deg_all[c * DPC : (c + 1) * DPC]
        rank = np.argsort(deg, kind="stable")[::-1]
        perm = np.full(nt * 128, -1, dtype=np.int64)
        perm[:DPC] = rank
        perms.append(perm)
        d = np.where(perm >= 0, deg[np.clip(perm, 0, DPC - 1)], 0)
        tmax = np.maximum(tmax, d.reshape(nt, 128).max(1))
    n_tiles = nt

    # per-position max degree across cores (non-increasing: each core is
    # degree-desc sorted)
    npos = n_tiles * 128
    dpos = np.zeros(npos, np.int64)
    for c in range(N_CORES):
        deg = deg_all[c * DPC : (c + 1) * DPC]
        dp = np.zeros(npos, np.int64)
        dp[:DPC] = np.sort(deg)[::-1]
        dpos = np.maximum(dpos, dp)

    # greedy variable-cap blocks: cap = degree of the first (largest) dst of
    # the block; blocks are free to cross tile boundaries but not PSUM
    # chunk boundaries
    CHP = CHT * 128
    blk_of_pos = np.zeros(npos, np.int64)
    g_of_pos = np.zeros(npos, np.int64)
    cap_of_pos = np.zeros(npos, np.int64)
    blocks = []  # (ck, gg, dcol, cap)
    caps_used = {}
    q = 0
    while q < npos:
        chunk_end = min((q // CHP + 1) * CHP, npos)
        cap = max(1, int(dpos[q]))
        G = 128 // cap
        gg = min(G, chunk_end - q)
        caps_used.setdefault(cap, G)
        b = len(blocks)
        blocks.append((q // CHP, gg, q % CHP, cap))
        blk_of_pos[q : q + gg] = b
        g_of_pos[q : q + gg] = np.arange(gg)
        cap_of_pos[q : q + gg] = cap
        q += gg
    tot_blk = len(blocks)
    n_chunks = -(-npos // CHP)
    chunk_cols = [min(CHP, npos - ck * CHP) for ck in range(n_chunks)]

    caps = []
    gcol = 0
    capmap = {}
    for cap in sorted(caps_used):
        G = caps_used[cap]
        caps.append((cap, G, gcol))
        capmap[cap] = gcol
        gcol += G
    ones_cols = gcol
    blocks = [(ck, gg, dcol, capmap[cap]) for (ck, gg, dcol, cap) in blocks]

    # pack block runs into calls of <= BMAX blocks (ramped starts)
    calls = []
    b0 = 0
    while b0 < tot_blk:
        lim = (24, 48)[len(calls)] if len(calls) < 2 else BMAX
        nblk = min(lim, tot_blk - b0)
        calls.append((b0, nblk, None))
        b0 += nblk

    # --- static engine planner: split the multiply DVE vs Pool per call ---
    planned = []
    flushA = 2 * (FIX_ACT + 512 * ACT_EL) / CHT
    flushD = (FIX_DVE + 512 * DVE1X + FIX_DVE + 512 * DVE2X) / CHT
    ncalls_total = len(calls)
    for cidx, (b0, nblk, _) in enumerate(calls):
        pool_el = POOL_EL * (1.7 if cidx >= ncalls_total - 2 else 1.0)
        ntl = 4 * sum(
            1
            for bi in range(b0, b0 + nblk)
            if bi + 1 == tot_blk or blocks[bi + 1][0] != blocks[bi][0]
        )
        actT = 2 * FIX_ACT + 2 * nblk * ACT_EL + ntl * flushA  # e1+exp+flush
        dveT = FIX_DVE + nblk * DVE1X + ntl * flushD  # epre + flush
        poolT = 0.0
        parts = []
        pb = 0
        while pb < nblk:
            nb = min(12, nblk - pb)
            a_x = actT + FIX_ACT + nb * EXPW * ACT_EL
            d_x = dveT + (64 // EXPW) * FIX_DVE + nb * 64 * DVE2X
            p_y = poolT + FIX_POOL + nb * 64 * pool_el
            if max(a_x, d_x, poolT) <= max(actT, dveT, p_y):
                actT, dveT = a_x, d_x
                parts.append((pb, nb, 0))
            else:
                poolT = p_y
                parts.append((pb, nb, 1))
            pb += nb
        planned.append((b0, nblk, parts))
    calls = planned

    # --- per-core cell maps ----------------------------------------------
    cell_srcs, cell_dsts = [], []
    for c in range(N_CORES):
        perm = perms[c]
        real = perm >= 0
        pos_of_dst = np.empty(DPC, np.int64)
        pos_of_dst[perm[real]] = np.flatnonzero(real)

        lo, hi = starts_all[c * DPC], starts_all[(c + 1) * DPC]
        e_src = src_s[lo:hi].astype(np.int64)
        e_dstl = dst_s[lo:hi] - c * DPC
        e_k = k_s[lo:hi]
        pos = pos_of_dst[e_dstl]
        e_p = (g_of_pos[pos] * cap_of_pos[pos] + e_k).astype(np.int64)
        e_b = blk_of_pos[pos]
        S = np.full((128, tot_blk), -1, np.int32)
        Dst = np.full((128, tot_blk), -1, np.int32)
        S[e_p, e_b] = e_src
        Dst[e_p, e_b] = c * DPC + e_dstl
        cell_srcs.append(S)
        cell_dsts.append(Dst)

    return Schedule(
        n_tiles, tot_blk, chunk_cols, blocks, calls, caps, ones_cols,
        perms, cell_srcs, cell_dsts,
    )


# ---------------------------------------------------------------- prog A
def build_progA(n_loc=DPC, in_c=IN_C, f=F):
    # one fused matmul per 512-col chunk: lhsT = [W1 | W1@A1cat] so Wh and
    # both attention scalars come out of a single PSUM tile / copy
    nc = bacc.Bacc("TRN2", target_bir_lowering=False, debug=False, num_devices=N_CORES)
    xT = nc.dram_tensor("xT", [in_c, n_loc], F16, kind="ExternalInput").ap()
    WP = nc.dram_tensor("WP", [in_c, f + 2], F16, kind="ExternalInput").ap()
    bP = nc.dram_tensor("bP", [f + 2, 1], F32, kind="ExternalInput").ap()
    combA = nc.dram_tensor("combA", [f + 2, n_loc], F16, kind="ExternalOutput").ap()

    with tile.TileContext(nc) as tc:
        with tc.tile_pool(name="sb", bufs=1) as pool, tc.tile_pool(
            name="ps", bufs=4, space="PSUM"
        ) as pps, tc.tile_pool(name="sb2", bufs=3) as pool2:
            WP_sb = pool.tile([in_c, f + 2], F16)
            nc.sync.dma_start(out=WP_sb[:], in_=WP[:, :])
            bP_sb = pool.tile([f + 2, 1], F32)
            nc.sync.dma_start(out=bP_sb[:], in_=bP[:, :])
            xT_sb = pool.tile([in_c, n_loc], F16)
            XCH = 3125
            for x0 in range(0, n_loc, XCH):
                xc = min(XCH, n_loc - x0)
                nc.sync.dma_start(
                    out=xT_sb[:, x0 : x0 + xc], in_=xT[:, x0 : x0 + xc]
                )

            CH = 512
            GRP = 8
            wh_g = None
            for c0 in range(0, n_loc, CH):
                ch = min(CH, n_loc - c0)
                gi = (c0 // CH) % GRP
                if gi == 0:
                    wh_g = pool2.tile([f + 2, GRP * CH], F16, tag="whg")
                ps_w = pps.tile([f + 2, CH], F32, space="PSUM")
                nc.tensor.matmul(
                    out=ps_w[:, :ch],
                    lhsT=WP_sb[:],
                    rhs=xT_sb[:, c0 : c0 + ch],
                    start=True,
                    stop=True,
                )
                if (c0 // CH) % 2 == 0:
                    # alternate the PSUM->SBUF copy between Act and DVE
                    nc.scalar.activation(
                        out=wh_g[:, gi * CH : gi * CH + ch],
                        in_=ps_w[:, :ch],
                        func=mybir.ActivationFunctionType.Identity,
                        bias=bP_sb[:],
                    )
                else:
                    nc.vector.tensor_scalar(
                        out=wh_g[:, gi * CH : gi * CH + ch],
                        in0=ps_w[:, :ch],
                        scalar1=bP_sb[:, 0:1],
                        scalar2=None,
                        op0=mybir.AluOpType.add,
                    )
                if gi == GRP - 1 or c0 + ch >= n_loc:
                    g0 = (c0 // CH // GRP) * GRP * CH
                    gl = c0 + ch - g0
                    nc.sync.dma_start(
                        out=combA[:, g0 : g0 + gl], in_=wh_g[:, :gl]
                    )
    nc.compile()
    return nc


# ---------------------------------------------------------------- prog B
def build_progB(sched: Schedule, f=F):
    NT = sched.n_tiles
    TB = sched.tot_blk
    nc = bacc.Bacc("TRN2", target_bir_lowering=False, debug=False, num_devices=N_CORES)
    tbl = nc.dram_tensor("tbl", [128, TB * (f + 1)], F16, kind="ExternalInput").ap()
    sjsi = nc.dram_tensor("sjsi", [128, 2 * TB], F16, kind="ExternalInput").ap()
    # rows 0-63 x cols 0-65 = [Wn | Wn@As]; col 66 = bias (66 rows)
    wp_d = nc.dram_tensor("wpack", [f + 2, f + 3], F32, kind="ExternalInput").ap()
    ones_d = nc.dram_tensor(
        "ones", [128, sched.ones_cols + f], F16, kind="ExternalInput"
    ).ap()
    shf_d = nc.dram_tensor("shiftv", [128, 1], F32, kind="ExternalInput").ap()
    comb = nc.dram_tensor("comb", [f + 2, NT * 128], F16, kind="ExternalOutput").ap()

    AF = mybir.ActivationFunctionType
    OP = mybir.AluOpType
    P = f + 1  # 65: per-block pitch

    def v(ap, dims, off=0):
        return dataclasses.replace(
            ap,
            ap=[list(ap.ap[0])] + [list(d) for d in dims],
            offset=ap.offset + off,
        )

    with tile.TileContext(nc) as tc:
        with tc.tile_pool(name="const", bufs=1) as pc, tc.tile_pool(
            name="gat", bufs=5
        ) as pg, tc.tile_pool(name="exw", bufs=4) as px, tc.tile_pool(
            name="work", bufs=5
        ) as pw, tc.tile_pool(name="ps", bufs=2, space="PSUM") as pps, tc.tile_pool(
            name="psb", bufs=3, space="PSUM"
        ) as ppsb, tc.tile_pool(name="ep", bufs=3) as pep:
            wp_sb = pc.tile([f + 2, f + 3], F32)
            WC_sb = pc.tile([f, f + 2], F16)
            bias_sb = wp_sb[:, f + 2 : f + 3]
            ones_sb = pc.tile([128, sched.ones_cols + f], F16)
            shf_sb = pc.tile([128, 1], F32)
            sjsi_sb = pc.tile([128, 2 * TB], F16)
            nc.sync.dma_start(out=shf_sb[:], in_=shf_d[:, :])
            nc.sync.dma_start(out=ones_sb[:], in_=ones_d[:, :])
            nc.sync.dma_start(out=sjsi_sb[:], in_=sjsi[:, :])

            def emit_consts():
                nc.sync.dma_start(out=wp_sb[:], in_=wp_d[:, :])
                nc.vector.tensor_scalar(
                    out=WC_sb[:],
                    in0=wp_sb[:f, : f + 2],
                    scalar1=1.0,
                    scalar2=None,
                    op0=OP.mult,
                )

            ps_ch = None

            def flush_chunk(ck, cols, ps_ch):
                # leaky rides the PSUM->SBUF copy; positive rden scales
                # commute with leaky-relu so normalize happens after
                hTL = pep.tile([f, CHT * 128], F16, tag="hTL")
                nc.scalar.activation(
                    out=hTL[:, :cols],
                    in_=ps_ch[0:f, :cols],
                    func=AF.Prelu,
                    alpha=ALPHA,
                )
                rdn = pep.tile([1, CHT * 128], F16, tag="rdn")
                with nc.allow_low_precision(
                    reason="rden in f16: SHIFT_TARGET=6 bounds den to "
                    "[7e-5, 1.5e4], all normal-range f16"
                ):
                    nc.vector.reciprocal(
                        out=rdn[:, :cols], in_=ps_ch[f : f + 1, :cols]
                    )
                ps_r = pps.tile([f, CHT * 128], F32, tag="psr", space="PSUM")
                nc.tensor.matmul(
                    out=ps_r[:, :cols],
                    lhsT=ones_sb[0:1, sched.ones_cols : sched.ones_cols + f],
                    rhs=rdn[:, :cols],
                    start=True,
                    stop=True,
                )
                ht = pep.tile([f, CHT * 128], F16, tag="ht")
                nc.vector.tensor_tensor(
                    out=ht[:, :cols],
                    in0=hTL[:, :cols],
                    in1=ps_r[:, :cols],
                    op=OP.mult,
                )
                ps_c = pps.tile([f + 2, CHT * 128], F32, tag="psc", space="PSUM")
                nc.tensor.matmul(
                    out=ps_c[:, :cols],
                    lhsT=WC_sb[:],
                    rhs=ht[:, :cols],
                    start=True,
                    stop=True,
                )
                c_sb = pep.tile([f + 2, CHT * 128], F16, tag="csb")
                nc.scalar.activation(
                    out=c_sb[:, :cols],
                    in_=ps_c[:, :cols],
                    func=AF.Identity,
                    bias=bias_sb,
                )
                nc.sync.dma_start(
                    out=comb[:, ck * CHT * 128 : ck * CHT * 128 + cols],
                    in_=c_sb[:, :cols],
                )

            def stage1(ci):
                b0, nblk, parts = sched.calls[ci]
                rhs = pg.tile([128, BMAX * P], F16, tag="rhs")
                nc.sync.dma_start(
                    out=rhs[:, : nblk * P], in_=tbl[:, b0 * P : (b0 + nblk) * P]
                )
                epre = pw.tile([128, BMAX], F32, tag="epre")
                nc.vector.tensor_tensor(
                    out=epre[:, :nblk],
                    in0=sjsi_sb[:, b0 : b0 + nblk],
                    in1=sjsi_sb[:, TB + b0 : TB + b0 + nblk],
                    op=OP.add,
                )
                e1 = pw.tile([128, BMAX], F32, tag="e1")
                nc.scalar.activation(
                    out=e1[:, :nblk], in_=epre[:, :nblk], func=AF.Prelu, alpha=ALPHA
                )
                # exp lands in the col-64 holes of the streamed cells
                nc.scalar.activation(
                    out=v(rhs[:], [(P, nblk)], off=f),
                    in_=e1[:, :nblk],
                    func=AF.Exp,
                    bias=shf_sb[:],
                )
                exw = None
                for (pb, nb, m_pool) in parts:
                    if not m_pool:
                        if exw is None:
                            exw = px.tile([128, BMAX * EXPW], F16, tag="exw")
                        nc.scalar.activation(
                            out=v(
                                exw[:], [(EXPW, nb), (1, EXPW)], off=pb * EXPW
                            ),
                            in_=v(e1[:], [(1, nb), (0, EXPW)], off=pb),
                            func=AF.Exp,
                            bias=shf_sb[:],
                        )
                return rhs, exw

            def stage2a(ci, rhs, exw):
                b0, nblk, parts = sched.calls[ci]
                for (pb, nb, m_pool) in parts:
                    if m_pool:
                        nc.gpsimd.tensor_tensor(
                            out=v(rhs[:], [(P, nb), (1, f)], off=pb * P),
                            in0=v(rhs[:], [(P, nb), (1, f)], off=pb * P),
                            in1=v(rhs[:], [(P, nb), (0, f)], off=pb * P + f),
                            op=OP.mult,
                        )
                for (pb, nb, m_pool) in parts:
                    if not m_pool:
                        for q in range(0, f, EXPW):
                            nc.vector.tensor_tensor(
                                out=v(rhs[:], [(P, nb), (1, EXPW)], off=pb * P + q),
                                in0=v(rhs[:], [(P, nb), (1, EXPW)], off=pb * P + q),
                                in1=v(
                                    exw[:], [(EXPW, nb), (1, EXPW)], off=pb * EXPW
                                ),
                                op=OP.mult,
                            )

            def stage2b(ci, rhs, exw):
                nonlocal ps_ch
                b0, nblk, parts = sched.calls[ci]
                done = []
                for bi in range(nblk):
                    (ck, gg, dcol, gcol0) = sched.blocks[b0 + bi]
                    if dcol == 0:
                        ps_ch = ppsb.tile(
                            [f + 1, CHT * 128], F32, tag="psch", space="PSUM"
                        )
                    nc.tensor.matmul(
                        out=ps_ch[:, dcol : dcol + gg],
                        lhsT=rhs[:, bi * P : (bi + 1) * P],
                        rhs=ones_sb[:, gcol0 : gcol0 + gg],
                        start=True,
                        stop=True,
                    )
                    if b0 + bi + 1 == TB or sched.blocks[b0 + bi + 1][0] != ck:
                        done.append((ck, sched.chunk_cols[ck], ps_ch))
                return done

            st1 = {}
            st2b = {}
            ncalls = len(sched.calls)
            for ci in range(ncalls + 4):
                if ci < ncalls:
                    st1[ci] = stage1(ci)
                    if ci == 0:
                        emit_consts()
                if 2 <= ci < ncalls + 2:
                    stage2a(ci - 2, *st1[ci - 2])
                if 3 <= ci < ncalls + 3:
                    st2b[ci - 3] = stage2b(ci - 3, *st1[ci - 3])
                if ci >= 4:
                    # flushes deferred one step so recip/norm never block
                    # the DVE stream while the chunk's matmuls still run
                    for (ck, cols, ps) in st2b[ci - 4]:
                        flush_chunk(ck, cols, ps)
    nc.compile()
    return nc


# ---------------------------------------------------------------- driver
_cache = {}
TRACE = False
LAST_HW_NS = []
LAST_RESULTS = []


def _run(nc, in_maps, cores):
    res = bass_utils.run_bass_kernel_spmd(nc, in_maps, core_ids=cores, trace=TRACE)
    if TRACE:
        LAST_RESULTS.append(res)
        if res.exec_time_ns:
            LAST_HW_NS.append(res.exec_time_ns)
    return res


def _get_schedule(edge_index):
    fp = hashlib.sha1(np.ascontiguousarray(edge_index)).hexdigest()
    key = ("sched", fp)
    if key not in _cache:
        _cache[key] = build_schedule(edge_index)
    return _cache[key]


def kernel(x, edge_index, W1, bW1, A1, bA1, W2, bW2, A2, bA2, Wfc, bfc):
    x = np.asarray(x, dtype=np.float32)
    edge_index = np.asarray(edge_index)
    W1 = np.asarray(W1, np.float32)
    bW1 = np.asarray(bW1, np.float32)
    A1 = np.asarray(A1, np.float32)
    bA1 = np.asarray(bA1, np.float32)
    W2 = np.asarray(W2, np.float32)
    bW2 = np.asarray(bW2, np.float32)
    A2 = np.asarray(A2, np.float32)
    bA2 = np.asarray(bA2, np.float32)
    Wfc = np.asarray(Wfc, np.float32)
    bfc = np.asarray(bfc, np.float32)

    sched = _get_schedule(edge_index)
    cores = list(range(N_CORES))

    if "A" not in _cache:
        _cache["A"] = build_progA()
    ncA = _cache["A"]
    inA = []
    x16 = x.astype(np.float16)
    A1cat = np.concatenate([A1[:F], A1[F:]], axis=1)  # [64, 2]
    WP = np.concatenate([W1, W1 @ A1cat], axis=1).astype(np.float16)  # [128, 66]
    bP = np.concatenate([bW1, bW1 @ A1cat]).reshape(F + 2, 1).astype(np.float32)
    for c in cores:
        xT = np.ascontiguousarray(x16[c * DPC : (c + 1) * DPC].T)
        inA.append({"xT": xT, "WP": WP, "bP": bP})
    LAST_HW_NS.clear()
    LAST_RESULTS.clear()
    resA = _run(ncA, inA, cores)
    cA = np.concatenate([resA.results[c]["combA"] for c in cores], axis=1)
    wh16 = np.ascontiguousarray(cA[:F].T)  # [N, 64] f16
    si_full = cA[F].astype(np.float32)
    sj_full = cA[F + 1].astype(np.float32)

    key = ("B", sched.n_tiles, sched.tot_blk, tuple(s[0] for s in sched.caps))
    if key not in _cache:
        _cache[key] = build_progB(sched)
    ncB = _cache[key]
    NT = sched.n_tiles
    TB = sched.tot_blk

    ones_host = np.zeros((128, sched.ones_cols + F), np.float16)
    for (cap, G, gcol0) in sched.caps:
        for g in range(G):
            ones_host[g * cap : (g + 1) * cap, gcol0 + g] = 1
    ones_host[0, sched.ones_cols :] = 1  # rden partition-broadcast columns

    def launch_B(wh16_full, si_f, sj_f, bA, Wn, bWn, An):
        bA0 = np.float32(bA.reshape(-1)[0])
        WnAs = Wn @ An  # [64, 2]
        wpack = np.zeros((F + 2, F + 3), np.float32)
        wpack[:F, :F] = Wn
        wpack[:F, F : F + 2] = WnAs
        wpack[:F, F + 2] = bWn.reshape(F)
        wpack[F : F + 2, F + 2] = bWn @ An
        inB = []
        shift = np.float32(
            max(0.0, float(si_f.max() + sj_f.max() + bA0) - SHIFT_TARGET)
        )
        for c in cores:
            ss = sched.cell_src[c]
            dd = sched.cell_dst[c]
            m = ss >= 0
            t = np.zeros((128, TB, F + 1), np.float16)
            t[:, :, :F] = wh16_full[np.clip(ss, 0, N_NODES - 1)]
            t[:, :, :F][~m] = 0
            sj_cell = np.full((128, TB), np.float32(PAD_SJ))
            sj_cell[m] = sj_f[ss[m]]
            si_cell = np.zeros((128, TB), np.float32)
            si_cell[m] = si_f[dd[m]] + bA0
            sjsi = np.empty((128, 2 * TB), np.float16)
            sjsi[:, :TB] = sj_cell
            sjsi[:, TB:] = si_cell
            inB.append(
                {
                    "tbl": np.ascontiguousarray(t.reshape(128, TB * (F + 1))),
                    "sjsi": sjsi,
                    "wpack": wpack,
                    "ones": ones_host,
                    "shiftv": np.full((128, 1), -shift, np.float32),
                }
            )
        res = _run(ncB, inB, cores)
        whn = np.zeros((N_NODES, F), np.float16)
        sn_i = np.zeros(N_NODES, np.float32)
        sn_j = np.zeros(N_NODES, np.float32)
        for c in cores:
            perm = sched.perms[c]
            real = perm >= 0
            gids = c * DPC + perm[real]
            cb = res.results[c]["comb"]
            whn[gids] = cb[:F].T[real]
            sn_i[gids] = cb[F].astype(np.float32)[real]
            sn_j[gids] = cb[F + 1].astype(np.float32)[real]
        return whn, sn_i, sn_j

    As2 = np.ascontiguousarray(np.concatenate([A2[:F], A2[F:]], axis=1))
    wh2, si2, sj2 = launch_B(wh16, si_full, sj_full, bA1, W2, bW2, As2)
    out, _, _ = launch_B(wh2, si2, sj2, bA2, Wfc, bfc, np.zeros((F, 2), np.float32))
    return out.astype(np.float32)
